# revision 20
# baseline (speedup 1.0000x reference)
"""HeteroGNN (2-layer hetero GCN) Trainium2 kernel, 8-core SPMD.

Strategy: destination-sharded. Each core owns 6250 drug + 6250 dis nodes.
Feature tables (bf16 rows) live in per-core HBM; edge gathers use
dma_gather (custom SWDGE row gather); scatter-add is done as one-hot
"Msel" matmuls accumulating in PSUM (edges chunked 128 at a time, each
chunk's destinations confined to a 32-wide bin so PSUM offsets are
program constants shared by all cores). Layer-1 output slices are
exchanged with two AllGather collectives, then layer 2 + final linear.
All graph preprocessing (degrees, norms, chunking, padding to the
max-over-cores schedule) happens on host in numpy.
"""

import numpy as np
import ml_dtypes

import sys

for _p in ("/opt/trn_rl_repo",):
    if _p not in sys.path:
        sys.path.insert(0, _p)

import concourse.bass as bass
import concourse.mybir as mybir
from concourse import tile
from concourse.bass_utils import run_bass_kernel_spmd

BF16 = mybir.dt.bfloat16
F32 = mybir.dt.float32
I16 = mybir.dt.int16


class Cfg:
    def __init__(self, n=50000, e=800000, ncores=8, win=256, binw=32, group=1):
        self.N = n              # nodes per type
        self.E = e              # edges per relation
        self.NC = ncores
        self.S = n // ncores    # dst nodes per core per type
        self.WIN = win          # dsts per PSUM window
        self.BINW = binw        # dsts per bin (fixed psum offset granularity)
        self.GROUP = group      # windows per gather call
        self.NW = (self.S + win - 1) // win   # windows per type
        self.NG = (self.NW + group - 1) // group
        self.HALF = n // 2      # rows per gather half-table (int16 idx limit)
        assert self.HALF <= 32768
        self.D = 128
        self.OUT = 64

    def win_size(self, w):
        return min(self.WIN, self.S - w * self.WIN)

    def nbins(self, w):
        ws = self.win_size(w)
        return (ws + self.BINW - 1) // self.BINW


# relations per dst type: (reference rel index, src_is_dis)
# drug dst: rel 0 (dd, src drug), rel 3 (sd, src dis)
# dis  dst: rel 1 (ss, src dis),  rel 2 (ds, src drug)
REL_OF_T = {0: [(0, 0), (3, 1)], 1: [(1, 1), (2, 0)]}
SELF_LOOP = {0: True, 1: True, 2: False, 3: False}


def _prep_graph(cfg, edge_arrays):
    """edge_arrays: dict rel_idx -> (row, col) int64 full edge lists.
    Returns (meta, per_core) where meta is the SPMD-uniform schedule and
    per_core[c] = dict(idx=int16 [128, ICOLS], msel=f32 [128, MCOLS])."""
    N, S, WIN, BINW, NC = cfg.N, cfg.S, cfg.WIN, cfg.BINW, cfg.NC

    # chunks[(t, w, r, h, b)][core] = list of (idx128 array, dloc array, norm array)
    group_chunks = {}
    for t in (0, 1):
        for ri, (r, src_dis) in enumerate(REL_OF_T[t]):
            row, col = edge_arrays[r]
            if SELF_LOOP[r]:
                sl = np.arange(N, dtype=np.int64)
                row = np.concatenate([row, sl])
                col = np.concatenate([col, sl])
            deg_s = np.bincount(row, minlength=N).astype(np.float64)
            deg_d = np.bincount(col, minlength=N).astype(np.float64)
            norm = (deg_s[row] ** -0.5 * deg_d[col] ** -0.5).astype(np.float32)
            core = col // S
            d_loc = col % S
            w = d_loc // WIN
            b = (d_loc % WIN) // BINW
            h = row // cfg.HALF
            idx16 = (row % cfg.HALF).astype(np.int16)
            # group key: (core, w, b, h)
            nb_max = (WIN + BINW - 1) // BINW
            key = ((core * cfg.NW + w) * nb_max + b) * 2 + h
            order = np.argsort(key, kind="stable")
            key_s = key[order]
            uk, starts = np.unique(key_s, return_index=True)
            starts = list(starts) + [len(key_s)]
            for gi, k in enumerate(uk):
                sl_ = order[starts[gi]:starts[gi + 1]]
                kk = int(k)
                hh = kk % 2
                kk //= 2
                bb = kk % nb_max
                kk //= nb_max
                ww = kk % cfg.NW
                cc = kk // cfg.NW
                gkey = (t, ww, ri, hh, bb)
                group_chunks.setdefault(gkey, {c: [] for c in range(NC)})
                lst = group_chunks[gkey][cc]
                for s0 in range(0, len(sl_), 128):
                    ee = sl_[s0:s0 + 128]
                    lst.append((idx16[ee], (d_loc[ee] % WIN) % BINW, norm[ee]))

    # C_max per slot key
    cmax = {}
    for gkey, bycore in group_chunks.items():
        cmax[gkey] = max(len(v) for v in bycore.values())

    # Build uniform schedule.
    # calls: per (t, g, ri, h): list of chunk slot keys in order (w asc, b asc, dup)
    calls = []           # (t, g, ri, h, src_dis, idx_col_off, nchunks)
    call_lookup = {}     # (t, g, ri, h) -> call index
    icol = 0
    for t in (0, 1):
        for g in range(cfg.NG):
            for ri in range(2):
                src_dis = REL_OF_T[t][ri][1]
                for h in (0, 1):
                    nch = 0
                    for w in range(g * cfg.GROUP, min((g + 1) * cfg.GROUP, cfg.NW)):
                        for b in range(cfg.nbins(w)):
                            nch += cmax.get((t, w, ri, h, b), 0)
                    call_lookup[(t, g, ri, h)] = len(calls)
                    calls.append(dict(t=t, g=g, ri=ri, h=h, src_dis=src_dis,
                                      icol=icol, nchunks=nch))
                    icol += nch * 8
    ICOLS = max(icol, 8)

    # windows: per (t, w): msel col offset + chunk list
    windows = {}
    mcol = 0
    for t in (0, 1):
        for w in range(cfg.NW):
            wch = []   # (ri, h, j_in_call, psum_off)
            # j_in_call accumulators per (ri, h) for this group
            for ri in range(2):
                for h in (0, 1):
                    j = 0
                    g = w // cfg.GROUP
                    for w2 in range(g * cfg.GROUP, w):
                        for b in range(cfg.nbins(w2)):
                            j += cmax.get((t, w2, ri, h, b), 0)
                    for b in range(cfg.nbins(w)):
                        for d in range(cmax.get((t, w, ri, h, b), 0)):
                            wch.append((ri, h, j, b * BINW))
                            j += 1
            windows[(t, w)] = dict(mcol=mcol, chunks=wch)
            mcol += len(wch) * BINW
    MCOLS = max(mcol, BINW)

    meta = dict(calls=calls, call_lookup=call_lookup, windows=windows,
                ICOLS=ICOLS, MCOLS=MCOLS)

    # Per-core data arrays
    per_core = []
    for c in range(NC):
        idxa = np.zeros((16, ICOLS), np.int16)
        msel = np.zeros((128, MCOLS), np.float32)
        for call in calls:
            t, g, ri, h = call["t"], call["g"], call["ri"], call["h"]
            j = 0
            for w in range(g * cfg.GROUP, min((g + 1) * cfg.GROUP, cfg.NW)):
                for b in range(cfg.nbins(w)):
                    ck = group_chunks.get((t, w, ri, h, b))
                    lst = ck[c] if ck else []
                    for d in range(cmax.get((t, w, ri, h, b), 0)):
                        if d < len(lst):
                            ii, dd, nn = lst[d]
                            s0 = j * 128
                            sl = np.arange(s0, s0 + len(ii))
                            idxa[sl % 16, call["icol"] + sl // 16] = ii
                        j += 1
        # msel fill: walk windows
        for (t, w), wd in windows.items():
            dupc = {}
            for ci, (ri, h, jc, poff) in enumerate(wd["chunks"]):
                b = poff // BINW
                ck = group_chunks.get((t, w, ri, h, b))
                lst = ck[c] if ck else []
                d = dupc.get((ri, h, poff), 0)
                dupc[(ri, h, poff)] = d + 1
                if d < len(lst):
                    ii, dd, nn = lst[d]
                    m0 = wd["mcol"] + ci * BINW
                    msel[np.arange(len(ii)), m0 + dd] = nn
        idx_full = np.tile(idxa, (8, 1))
        per_core.append(dict(idx=idx_full,
                             msel=msel.astype(ml_dtypes.bfloat16)))
    return meta, per_core


def _build_program(cfg, meta):
    """Build the SPMD Bass program (same for all cores)."""
    from concourse import bacc

    NC, WIN, BINW = cfg.NC, cfg.WIN, cfg.BINW
    GSTEP = 8   # 1024 idxs per dma_gather: >1024 is over the Q7 scratch cap
    NQ = 4      # ucode MAX_SWDGE_QUEUES; rings process entries serially, so
                # spreading instructions over all 4 rings quadruples rate
    nc = bacc.Bacc("TRN2", target_bir_lowering=False, debug=False,
                   num_devices=NC, num_swdge_queues=NQ)

    # I/O
    xt = {}
    for sd, nm in ((0, "d"), (1, "s")):
        for h in (0, 1):
            xt[(sd, h)] = nc.dram_tensor(
                f"x_{nm}_h{h}", [cfg.HALF, 128], BF16, kind="ExternalInput")
    idx_d = nc.dram_tensor("idx", [128, meta["ICOLS"]], I16, kind="ExternalInput")
    msel_d = nc.dram_tensor("msel", [128, meta["MCOLS"]], BF16, kind="ExternalInput")
    wts_d = nc.dram_tensor("wts", [2, 2, 2, 128, 128], BF16, kind="ExternalInput")
    linwt_d = nc.dram_tensor("linwt", [128, cfg.OUT], BF16, kind="ExternalInput")
    bias1_d = nc.dram_tensor("bias1", [2, 128, 2 * 128], F32, kind="ExternalInput")
    bias2_d = nc.dram_tensor("bias2", [2, 128, 1], F32, kind="ExternalInput")
    linb_d = nc.dram_tensor("linb", [128, 2 * cfg.OUT], F32, kind="ExternalInput")
    out_d = nc.dram_tensor("out", [2 * cfg.S, cfg.OUT], F32, kind="ExternalOutput")

    z_loc = [nc.dram_tensor(f"z_loc{t}", [cfg.S, 128], BF16) for t in (0, 1)]
    z_full = [nc.dram_tensor(f"z_full{t}", [cfg.N, 128], BF16,
                             addr_space="Shared") for t in (0, 1)]
    z_hi = [nc.dram_tensor(f"z_hi{t}", [cfg.HALF, 128], BF16) for t in (0, 1)]

    calls, windows = meta["calls"], meta["windows"]
    call_lookup = meta["call_lookup"]

    gctr = [0]

    with tile.TileContext(nc) as tc:
        cpool = tc.alloc_tile_pool(name="const", bufs=1)
        gpool = tc.alloc_tile_pool(name="gather", bufs=6)
        ipool = tc.alloc_tile_pool(name="idx", bufs=6)
        mpool = tc.alloc_tile_pool(name="msel", bufs=3)
        apool = tc.alloc_tile_pool(name="aggs", bufs=2)
        zpool = tc.alloc_tile_pool(name="z", bufs=2)
        pagg = tc.alloc_tile_pool(name="pagg", bufs=2, space="PSUM")
        ptrf = tc.alloc_tile_pool(name="ptrf", bufs=2, space="PSUM")
        pfin = tc.alloc_tile_pool(name="pfin", bufs=2, space="PSUM")

        # constants to SBUF
        wsb = {}
        for l in (0, 1):
            for t in (0, 1):
                for ri in (0, 1):
                    w = cpool.tile([128, 128], BF16, tag=f"w{l}{t}{ri}", name=f"w_{l}{t}{ri}")
                    nc.sync.dma_start(w[:], wts_d[l, t, ri])
                    wsb[(l, t, ri)] = w
        linwt = cpool.tile([128, cfg.OUT], BF16, tag="linwt")
        nc.sync.dma_start(linwt[:], linwt_d[:])
        b1 = {}
        b2 = {}
        for t in (0, 1):
            b1[t] = cpool.tile([128, 256], F32, tag=f"b1{t}", name=f"b1_{t}")
            nc.sync.dma_start(b1[t][:], bias1_d[t])
            b2[t] = cpool.tile([128, 1], F32, tag=f"b2{t}", name=f"b2_{t}")
            nc.sync.dma_start(b2[t][:], bias2_d[t])
        linb = cpool.tile([128, 2 * cfg.OUT], F32, tag="linb")
        nc.sync.dma_start(linb[:], linb_d[:])

        def gather_tables(l, sd):
            if l == 0:
                return [xt[(sd, 0)].ap(), xt[(sd, 1)].ap()]
            return [z_full[sd][0:cfg.HALF, :], z_hi[sd].ap()]

        def do_layer(l):
            gtiles = {}
            for t in (0, 1):
                for w in range(cfg.NW):
                    ws = cfg.win_size(w)
                    g = w // cfg.GROUP
                    if w % cfg.GROUP == 0:
                        for ri in range(2):
                            for h in (0, 1):
                                call = calls[call_lookup[(t, g, ri, h)]]
                                C = call["nchunks"]
                                if C == 0:
                                    gtiles[(ri, h)] = None
                                    continue
                                it = ipool.tile([128, C * 8], I16, tag=f"i{ri}{h}", name=f"it{ri}{h}")
                                nc.sync.dma_start(
                                    it[:], idx_d[:, call["icol"]:call["icol"] + C * 8])
                                gt = gpool.tile([128, C, 128], BF16, tag=f"g{ri}{h}", name=f"gt{ri}{h}")
                                src = gather_tables(l, call["src_dis"])[h]
                                for j0 in range(0, C, GSTEP):
                                    cs = min(GSTEP, C - j0)
                                    nc.gpsimd.dma_gather(
                                        gt[:, j0:j0 + cs, :], src,
                                        it[:, j0 * 8:(j0 + cs) * 8],
                                        cs * 128, cs * 128, 128,
                                        queue_num=gctr[0] % NQ)
                                    gctr[0] += 1
                                gtiles[(ri, h)] = gt
                    wd = windows[(t, w)]
                    nch = len(wd["chunks"])
                    if nch:
                        mt = mpool.tile([128, nch * BINW], BF16, tag="m", name="mt")
                        nc.sync.dma_start(
                            mt[:], msel_d[:, wd["mcol"]:wd["mcol"] + nch * BINW])
                    aggP = [pagg.tile([128, 256], F32, tag=f"agg{r}", name=f"aggP{r}") for r in (0, 1)]
                    nc.vector.memset(aggP[0][:], 0.0)
                    nc.vector.memset(aggP[1][:], 0.0)
                    last_of_r = {}
                    for ci, (ri, h, j, poff) in enumerate(wd["chunks"]):
                        last_of_r[ri] = ci
                    for ci, (ri, h, j, poff) in enumerate(wd["chunks"]):
                        gt = gtiles[(ri, h)]
                        nc.tensor.matmul(
                            aggP[ri][:, poff:poff + BINW],
                            gt[:, j, :],
                            mt[:, ci * BINW:(ci + 1) * BINW],
                            start=False, stop=(last_of_r[ri] == ci),
                            skip_group_check=True)
                    aggS = []
                    for r in (0, 1):
                        a = apool.tile([128, 256], BF16, tag=f"as{r}", name=f"aggS{r}")
                        nc.vector.tensor_copy(a[:, 0:ws], aggP[r][:, 0:ws])
                        aggS.append(a)
                    if l == 0:
                        outP = ptrf.tile([128, 256], F32, tag="tp")
                        nsub = (ws + 127) // 128
                        for j in range(nsub):
                            m = min(128, ws - j * 128)
                            nc.tensor.matmul(
                                outP[0:m, j * 128:j * 128 + 128],
                                aggS[0][:, j * 128:j * 128 + m],
                                wsb[(l, t, 0)][:], start=True, stop=False)
                            nc.tensor.matmul(
                                outP[0:m, j * 128:j * 128 + 128],
                                aggS[1][:, j * 128:j * 128 + m],
                                wsb[(l, t, 1)][:], start=False, stop=True)
                        tmp = zpool.tile([128, 256], F32, tag="tmp")
                        zsb = zpool.tile([128, 256], BF16, tag="zsb")
                        for j in range(nsub):
                            m = min(128, ws - j * 128)
                            sl = slice(j * 128, j * 128 + 128)
                            nc.vector.tensor_add(tmp[0:m, sl], outP[0:m, sl],
                                                 b1[t][0:m, sl])
                            nc.scalar.activation(
                                zsb[0:m, sl], tmp[0:m, sl],
                                mybir.ActivationFunctionType.Relu)
                            base = w * WIN + j * 128
                            nc.sync.dma_start(
                                z_loc[t][base:base + m, :], zsb[0:m, sl])
                    else:
                        z2P = ptrf.tile([128, 256], F32, tag="tp")
                        nc.tensor.matmul(z2P[:, 0:ws], wsb[(l, t, 0)][:],
                                         aggS[0][:, 0:ws], start=True, stop=False)
                        nc.tensor.matmul(z2P[:, 0:ws], wsb[(l, t, 1)][:],
                                         aggS[1][:, 0:ws], start=False, stop=True)
                        z2T = zpool.tile([128, 256], BF16, tag="z2t")
                        nc.scalar.activation(z2T[:, 0:ws], z2P[:, 0:ws],
                                             mybir.ActivationFunctionType.Relu,
                                             bias=b2[t][:])
                        fP = pfin.tile([128, 128], F32, tag="fp")
                        nsub = (ws + 127) // 128
                        for j in range(nsub):
                            m = min(128, ws - j * 128)
                            nc.tensor.matmul(
                                fP[0:m, j * 64:j * 64 + 64],
                                z2T[:, j * 128:j * 128 + m],
                                linwt[:], start=True, stop=True)
                        fo = zpool.tile([128, 128], F32, tag="fo")
                        for j in range(nsub):
                            m = min(128, ws - j * 128)
                            sl = slice(j * 64, j * 64 + 64)
                            nc.vector.tensor_add(fo[0:m, sl], fP[0:m, sl],
                                                 linb[0:m, sl])
                            base = t * cfg.S + w * WIN + j * 128
                            nc.sync.dma_start(out_d[base:base + m, :], fo[0:m, sl])

        stage = getattr(cfg, "stage", 2)
        do_layer(0)
        for t in (0, 1):
            if stage < 1:
                break
            nc.gpsimd.collective_compute(
                "AllGather", mybir.AluOpType.bypass,
                replica_groups=[list(range(NC))],
                ins=[z_loc[t].ap().opt()], outs=[z_full[t].ap().opt()])
        if stage >= 1:
            for t in (0, 1):
                nc.sync.dma_start(z_hi[t].ap(), z_full[t][cfg.HALF:2 * cfg.HALF, :])
        if stage >= 2:
            do_layer(1)

        for p in (pfin, ptrf, pagg, zpool, apool, mpool, ipool, gpool, cpool):
            p.release()

    nc.compile()
    return nc


def _make_inputs(cfg, per_core, x_drug, x_dis, Ws, bs, lin_w, lin_b):
    bf = ml_dtypes.bfloat16
    xb = {0: x_drug.astype(bf), 1: x_dis.astype(bf)}
    wts = np.zeros((2, 2, 2, 128, 128), np.float32)
    b1 = np.zeros((2, 128, 256), np.float32)
    b2 = np.zeros((2, 128, 1), np.float32)
    for l in (0, 1):
        for t in (0, 1):
            for ri in (0, 1):
                r = REL_OF_T[t][ri][0]
                wts[l, t, ri] = Ws[l, r]
            bsum = bs[l, REL_OF_T[t][0][0]] + bs[l, REL_OF_T[t][1][0]]
            if l == 0:
                b1[t] = np.tile(bsum[None, :], (128, 2))
            else:
                b2[t] = bsum[:, None].astype(np.float32)
    shared = {
        "wts": wts.astype(bf),
        "linwt": lin_w.T.astype(bf).copy(),
        "bias1": b1,
        "bias2": b2,
        "linb": np.tile(lin_b[None, :], (128, 2)).astype(np.float32),
    }
    for sd, nm in ((0, "d"), (1, "s")):
        for h in (0, 1):
            shared[f"x_{nm}_h{h}"] = np.ascontiguousarray(
                xb[sd][h * cfg.HALF:(h + 1) * cfg.HALF])
    in_maps = []
    for c in range(cfg.NC):
        m = dict(shared)
        m["idx"] = per_core[c]["idx"]
        m["msel"] = per_core[c]["msel"]
        in_maps.append(m)
    return in_maps


def run(cfg, x_drug, x_dis, eis, Ws, bs, lin_w, lin_b, trace=False):
    edge_arrays = {r: (eis[r][0].astype(np.int64), eis[r][1].astype(np.int64))
                   for r in range(4)}
    meta, per_core = _prep_graph(cfg, edge_arrays)
    nc = _build_program(cfg, meta)
    in_maps = _make_inputs(cfg, per_core, x_drug, x_dis, Ws, bs, lin_w, lin_b)
    res = run_bass_kernel_spmd(nc, in_maps, core_ids=list(range(cfg.NC)),
                               trace=trace)
    drug = np.zeros((cfg.N, cfg.OUT), np.float32)
    dis = np.zeros((cfg.N, cfg.OUT), np.float32)
    for c in range(cfg.NC):
        o = res.results[c]["out"]
        drug[c * cfg.S:(c + 1) * cfg.S] = o[:cfg.S]
        dis[c * cfg.S:(c + 1) * cfg.S] = o[cfg.S:]
    return (drug, dis), res


def kernel(x_drug, x_dis, ei_dd, ei_ss, ei_ds, ei_sd, Ws, bs, lin_w, lin_b):
    cfg = Cfg()
    eis = {0: np.asarray(ei_dd), 1: np.asarray(ei_ss),
           2: np.asarray(ei_ds), 3: np.asarray(ei_sd)}
    out, _ = run(cfg, np.asarray(x_drug), np.asarray(x_dis), eis,
                 np.asarray(Ws), np.asarray(bs),
                 np.asarray(lin_w), np.asarray(lin_b))
    return out



# revision 21
# speedup vs baseline: 1.0667x; 1.0667x over previous
"""HeteroGNN (2-layer hetero GCN) Trainium2 kernel, 8-core SPMD.

Strategy: destination-sharded. Each core owns 6250 drug + 6250 dis nodes.
Feature tables (bf16 rows) live in per-core HBM; edge gathers use
dma_gather (custom SWDGE row gather); scatter-add is done as one-hot
"Msel" matmuls accumulating in PSUM (edges chunked 128 at a time, each
chunk's destinations confined to a 32-wide bin so PSUM offsets are
program constants shared by all cores). Layer-1 output slices are
exchanged with two AllGather collectives, then layer 2 + final linear.
All graph preprocessing (degrees, norms, chunking, padding to the
max-over-cores schedule) happens on host in numpy.
"""

import numpy as np
import ml_dtypes

import sys

for _p in ("/opt/trn_rl_repo",):
    if _p not in sys.path:
        sys.path.insert(0, _p)

import concourse.bass as bass
import concourse.mybir as mybir
from concourse import tile
from concourse.bass_utils import run_bass_kernel_spmd

BF16 = mybir.dt.bfloat16
F32 = mybir.dt.float32
I16 = mybir.dt.int16


class Cfg:
    def __init__(self, n=50000, e=800000, ncores=8, win=256, binw=32, group=1):
        self.N = n              # nodes per type
        self.E = e              # edges per relation
        self.NC = ncores
        self.S = n // ncores    # dst nodes per core per type
        self.WIN = win          # dsts per PSUM window
        self.BINW = binw        # dsts per bin (fixed psum offset granularity)
        self.GROUP = group      # windows per gather call
        self.NW = (self.S + win - 1) // win   # windows per type
        self.NG = (self.NW + group - 1) // group
        self.HALF = n // 2      # rows per gather half-table (int16 idx limit)
        assert self.HALF <= 32768
        self.D = 128
        self.OUT = 64

    def win_size(self, w):
        return min(self.WIN, self.S - w * self.WIN)

    def nbins(self, w):
        ws = self.win_size(w)
        return (ws + self.BINW - 1) // self.BINW


# relations per dst type: (reference rel index, src_is_dis)
# drug dst: rel 0 (dd, src drug), rel 3 (sd, src dis)
# dis  dst: rel 1 (ss, src dis),  rel 2 (ds, src drug)
REL_OF_T = {0: [(0, 0), (3, 1)], 1: [(1, 1), (2, 0)]}
SELF_LOOP = {0: True, 1: True, 2: False, 3: False}


def _prep_graph(cfg, edge_arrays):
    """edge_arrays: dict rel_idx -> (row, col) int64 full edge lists.
    Returns (meta, per_core) where meta is the SPMD-uniform schedule and
    per_core[c] = dict(idx=int16 [128, ICOLS], msel=f32 [128, MCOLS])."""
    N, S, WIN, BINW, NC = cfg.N, cfg.S, cfg.WIN, cfg.BINW, cfg.NC

    # chunks[(t, w, r, h, b)][core] = list of (idx128 array, dloc array, norm array)
    group_chunks = {}
    for t in (0, 1):
        for ri, (r, src_dis) in enumerate(REL_OF_T[t]):
            row, col = edge_arrays[r]
            if SELF_LOOP[r]:
                sl = np.arange(N, dtype=np.int64)
                row = np.concatenate([row, sl])
                col = np.concatenate([col, sl])
            deg_s = np.bincount(row, minlength=N).astype(np.float64)
            deg_d = np.bincount(col, minlength=N).astype(np.float64)
            norm = (deg_s[row] ** -0.5 * deg_d[col] ** -0.5).astype(np.float32)
            core = col // S
            d_loc = col % S
            w = d_loc // WIN
            b = (d_loc % WIN) // BINW
            h = row // cfg.HALF
            idx16 = (row % cfg.HALF).astype(np.int16)
            # group key: (core, w, b, h)
            nb_max = (WIN + BINW - 1) // BINW
            key = ((core * cfg.NW + w) * nb_max + b) * 2 + h
            order = np.argsort(key, kind="stable")
            key_s = key[order]
            uk, starts = np.unique(key_s, return_index=True)
            starts = list(starts) + [len(key_s)]
            for gi, k in enumerate(uk):
                sl_ = order[starts[gi]:starts[gi + 1]]
                kk = int(k)
                hh = kk % 2
                kk //= 2
                bb = kk % nb_max
                kk //= nb_max
                ww = kk % cfg.NW
                cc = kk // cfg.NW
                gkey = (t, ww, ri, hh, bb)
                group_chunks.setdefault(gkey, {c: [] for c in range(NC)})
                lst = group_chunks[gkey][cc]
                for s0 in range(0, len(sl_), 128):
                    ee = sl_[s0:s0 + 128]
                    lst.append((idx16[ee], (d_loc[ee] % WIN) % BINW, norm[ee]))

    # C_max per slot key
    cmax = {}
    for gkey, bycore in group_chunks.items():
        cmax[gkey] = max(len(v) for v in bycore.values())

    # Build uniform schedule.
    # calls: per (t, g, ri, h): list of chunk slot keys in order (w asc, b asc, dup)
    calls = []           # (t, g, ri, h, src_dis, idx_col_off, nchunks)
    call_lookup = {}     # (t, g, ri, h) -> call index
    icol = 0
    for t in (0, 1):
        for g in range(cfg.NG):
            for ri in range(2):
                src_dis = REL_OF_T[t][ri][1]
                for h in (0, 1):
                    nch = 0
                    for w in range(g * cfg.GROUP, min((g + 1) * cfg.GROUP, cfg.NW)):
                        for b in range(cfg.nbins(w)):
                            nch += cmax.get((t, w, ri, h, b), 0)
                    call_lookup[(t, g, ri, h)] = len(calls)
                    calls.append(dict(t=t, g=g, ri=ri, h=h, src_dis=src_dis,
                                      icol=icol, nchunks=nch))
                    icol += nch * 8
    ICOLS = max(icol, 8)

    # windows: per (t, w): msel col offset + chunk list
    windows = {}
    mcol = 0
    for t in (0, 1):
        for w in range(cfg.NW):
            wch = []   # (ri, h, j_in_call, psum_off)
            # j_in_call accumulators per (ri, h) for this group
            for ri in range(2):
                for h in (0, 1):
                    j = 0
                    g = w // cfg.GROUP
                    for w2 in range(g * cfg.GROUP, w):
                        for b in range(cfg.nbins(w2)):
                            j += cmax.get((t, w2, ri, h, b), 0)
                    for b in range(cfg.nbins(w)):
                        for d in range(cmax.get((t, w, ri, h, b), 0)):
                            wch.append((ri, h, j, b * BINW))
                            j += 1
            windows[(t, w)] = dict(mcol=mcol, chunks=wch)
            mcol += len(wch) * BINW
    MCOLS = max(mcol, BINW)

    meta = dict(calls=calls, call_lookup=call_lookup, windows=windows,
                ICOLS=ICOLS, MCOLS=MCOLS)

    # Per-core data arrays
    per_core = []
    for c in range(NC):
        idxa = np.zeros((16, ICOLS), np.int16)
        msel = np.zeros((128, MCOLS), np.float32)
        for call in calls:
            t, g, ri, h = call["t"], call["g"], call["ri"], call["h"]
            j = 0
            for w in range(g * cfg.GROUP, min((g + 1) * cfg.GROUP, cfg.NW)):
                for b in range(cfg.nbins(w)):
                    ck = group_chunks.get((t, w, ri, h, b))
                    lst = ck[c] if ck else []
                    for d in range(cmax.get((t, w, ri, h, b), 0)):
                        if d < len(lst):
                            ii, dd, nn = lst[d]
                            s0 = j * 128
                            sl = np.arange(s0, s0 + len(ii))
                            idxa[sl % 16, call["icol"] + sl // 16] = ii
                        j += 1
        # msel fill: walk windows
        for (t, w), wd in windows.items():
            dupc = {}
            for ci, (ri, h, jc, poff) in enumerate(wd["chunks"]):
                b = poff // BINW
                ck = group_chunks.get((t, w, ri, h, b))
                lst = ck[c] if ck else []
                d = dupc.get((ri, h, poff), 0)
                dupc[(ri, h, poff)] = d + 1
                if d < len(lst):
                    ii, dd, nn = lst[d]
                    m0 = wd["mcol"] + ci * BINW
                    msel[np.arange(len(ii)), m0 + dd] = nn
        idx_full = np.tile(idxa, (8, 1))
        per_core.append(dict(idx=idx_full,
                             msel=msel.astype(ml_dtypes.bfloat16)))
    return meta, per_core


def _build_program(cfg, meta):
    """Build the SPMD Bass program (same for all cores)."""
    from concourse import bacc

    NC, WIN, BINW = cfg.NC, cfg.WIN, cfg.BINW
    GSTEP = 8   # 1024 idxs per dma_gather: >1024 is over the Q7 scratch cap
    NQ = 4      # ucode MAX_SWDGE_QUEUES; rings process entries serially, so
                # spreading instructions over all 4 rings quadruples rate
    nc = bacc.Bacc("TRN2", target_bir_lowering=False, debug=False,
                   num_devices=NC, num_swdge_queues=NQ)

    # I/O
    xt = {}
    for sd, nm in ((0, "d"), (1, "s")):
        for h in (0, 1):
            xt[(sd, h)] = nc.dram_tensor(
                f"x_{nm}_h{h}", [cfg.HALF, 128], BF16, kind="ExternalInput")
    idx_d = nc.dram_tensor("idx", [128, meta["ICOLS"]], I16, kind="ExternalInput")
    msel_d = nc.dram_tensor("msel", [128, meta["MCOLS"]], BF16, kind="ExternalInput")
    wts_d = nc.dram_tensor("wts", [2, 2, 2, 128, 128], BF16, kind="ExternalInput")
    linwt_d = nc.dram_tensor("linwt", [128, cfg.OUT], BF16, kind="ExternalInput")
    bias1_d = nc.dram_tensor("bias1", [2, 128, 2 * 128], F32, kind="ExternalInput")
    bias2_d = nc.dram_tensor("bias2", [2, 128, 1], F32, kind="ExternalInput")
    linb_d = nc.dram_tensor("linb", [128, 2 * cfg.OUT], F32, kind="ExternalInput")
    out_d = nc.dram_tensor("out", [2 * cfg.S, cfg.OUT], F32, kind="ExternalOutput")

    z_loc = [nc.dram_tensor(f"z_loc{t}", [cfg.S, 128], BF16) for t in (0, 1)]
    z_full = [nc.dram_tensor(f"z_full{t}", [cfg.N, 128], BF16,
                             addr_space="Shared") for t in (0, 1)]
    z_hi = [nc.dram_tensor(f"z_hi{t}", [cfg.HALF, 128], BF16) for t in (0, 1)]

    calls, windows = meta["calls"], meta["windows"]
    call_lookup = meta["call_lookup"]

    gctr = [0]

    with tile.TileContext(nc) as tc:
        cpool = tc.alloc_tile_pool(name="const", bufs=1)
        gpool = tc.alloc_tile_pool(name="gather", bufs=4)
        ipool = tc.alloc_tile_pool(name="idx", bufs=4)
        mpool = tc.alloc_tile_pool(name="msel", bufs=2)
        apool = tc.alloc_tile_pool(name="aggs", bufs=2)
        zpool = tc.alloc_tile_pool(name="z", bufs=2)
        pagg = tc.alloc_tile_pool(name="pagg", bufs=2, space="PSUM")
        ptrf = tc.alloc_tile_pool(name="ptrf", bufs=2, space="PSUM")
        pfin = tc.alloc_tile_pool(name="pfin", bufs=2, space="PSUM")

        # constants to SBUF
        wsb = {}
        for l in (0, 1):
            for t in (0, 1):
                for ri in (0, 1):
                    w = cpool.tile([128, 128], BF16, tag=f"w{l}{t}{ri}", name=f"w_{l}{t}{ri}")
                    nc.sync.dma_start(w[:], wts_d[l, t, ri])
                    wsb[(l, t, ri)] = w
        linwt = cpool.tile([128, cfg.OUT], BF16, tag="linwt")
        nc.sync.dma_start(linwt[:], linwt_d[:])
        b1 = {}
        b2 = {}
        for t in (0, 1):
            b1[t] = cpool.tile([128, 256], F32, tag=f"b1{t}", name=f"b1_{t}")
            nc.sync.dma_start(b1[t][:], bias1_d[t])
            b2[t] = cpool.tile([128, 1], F32, tag=f"b2{t}", name=f"b2_{t}")
            nc.sync.dma_start(b2[t][:], bias2_d[t])
        linb = cpool.tile([128, 2 * cfg.OUT], F32, tag="linb")
        nc.sync.dma_start(linb[:], linb_d[:])

        def gather_tables(l, sd):
            if l == 0:
                return [xt[(sd, 0)].ap(), xt[(sd, 1)].ap()]
            return [z_full[sd][0:cfg.HALF, :], z_hi[sd].ap()]

        def do_layer(l):
            gtiles = {}
            for t in (0, 1):
                for w in range(cfg.NW):
                    ws = cfg.win_size(w)
                    g = w // cfg.GROUP
                    if w % cfg.GROUP == 0:
                        for ri in range(2):
                            for h in (0, 1):
                                call = calls[call_lookup[(t, g, ri, h)]]
                                C = call["nchunks"]
                                if C == 0:
                                    gtiles[(ri, h)] = None
                                    continue
                                it = ipool.tile([128, C * 8], I16, tag=f"i{ri}{h}", name=f"it{ri}{h}")
                                nc.sync.dma_start(
                                    it[:], idx_d[:, call["icol"]:call["icol"] + C * 8])
                                gt = gpool.tile([128, C, 128], BF16, tag=f"g{ri}{h}", name=f"gt{ri}{h}")
                                src = gather_tables(l, call["src_dis"])[h]
                                for j0 in range(0, C, GSTEP):
                                    cs = min(GSTEP, C - j0)
                                    nc.gpsimd.dma_gather(
                                        gt[:, j0:j0 + cs, :], src,
                                        it[:, j0 * 8:(j0 + cs) * 8],
                                        cs * 128, cs * 128, 128,
                                        queue_num=gctr[0] % NQ)
                                    gctr[0] += 1
                                gtiles[(ri, h)] = gt
                    wd = windows[(t, w)]
                    nch = len(wd["chunks"])
                    if nch:
                        mt = mpool.tile([128, nch * BINW], BF16, tag="m", name="mt")
                        nc.sync.dma_start(
                            mt[:], msel_d[:, wd["mcol"]:wd["mcol"] + nch * BINW])
                    aggP = [pagg.tile([128, 256], F32, tag=f"agg{r}", name=f"aggP{r}") for r in (0, 1)]
                    nc.vector.memset(aggP[0][:], 0.0)
                    nc.vector.memset(aggP[1][:], 0.0)
                    last_of_r = {}
                    for ci, (ri, h, j, poff) in enumerate(wd["chunks"]):
                        last_of_r[ri] = ci
                    for ci, (ri, h, j, poff) in enumerate(wd["chunks"]):
                        gt = gtiles[(ri, h)]
                        nc.tensor.matmul(
                            aggP[ri][:, poff:poff + BINW],
                            gt[:, j, :],
                            mt[:, ci * BINW:(ci + 1) * BINW],
                            start=False, stop=(last_of_r[ri] == ci),
                            skip_group_check=True)
                    aggS = []
                    for r in (0, 1):
                        a = apool.tile([128, 256], BF16, tag=f"as{r}", name=f"aggS{r}")
                        nc.vector.tensor_copy(a[:, 0:ws], aggP[r][:, 0:ws])
                        aggS.append(a)
                    if l == 0:
                        outP = ptrf.tile([128, 256], F32, tag="tp")
                        nsub = (ws + 127) // 128
                        for j in range(nsub):
                            m = min(128, ws - j * 128)
                            nc.tensor.matmul(
                                outP[0:m, j * 128:j * 128 + 128],
                                aggS[0][:, j * 128:j * 128 + m],
                                wsb[(l, t, 0)][:], start=True, stop=False)
                            nc.tensor.matmul(
                                outP[0:m, j * 128:j * 128 + 128],
                                aggS[1][:, j * 128:j * 128 + m],
                                wsb[(l, t, 1)][:], start=False, stop=True)
                        tmp = zpool.tile([128, 256], F32, tag="tmp")
                        zsb = zpool.tile([128, 256], BF16, tag="zsb")
                        for j in range(nsub):
                            m = min(128, ws - j * 128)
                            sl = slice(j * 128, j * 128 + 128)
                            nc.vector.tensor_add(tmp[0:m, sl], outP[0:m, sl],
                                                 b1[t][0:m, sl])
                            nc.scalar.activation(
                                zsb[0:m, sl], tmp[0:m, sl],
                                mybir.ActivationFunctionType.Relu)
                            base = w * WIN + j * 128
                            nc.sync.dma_start(
                                z_loc[t][base:base + m, :], zsb[0:m, sl])
                    else:
                        z2P = ptrf.tile([128, 256], F32, tag="tp")
                        nc.tensor.matmul(z2P[:, 0:ws], wsb[(l, t, 0)][:],
                                         aggS[0][:, 0:ws], start=True, stop=False)
                        nc.tensor.matmul(z2P[:, 0:ws], wsb[(l, t, 1)][:],
                                         aggS[1][:, 0:ws], start=False, stop=True)
                        z2T = zpool.tile([128, 256], BF16, tag="z2t")
                        nc.scalar.activation(z2T[:, 0:ws], z2P[:, 0:ws],
                                             mybir.ActivationFunctionType.Relu,
                                             bias=b2[t][:])
                        fP = pfin.tile([128, 128], F32, tag="fp")
                        nsub = (ws + 127) // 128
                        for j in range(nsub):
                            m = min(128, ws - j * 128)
                            nc.tensor.matmul(
                                fP[0:m, j * 64:j * 64 + 64],
                                z2T[:, j * 128:j * 128 + m],
                                linwt[:], start=True, stop=True)
                        fo = zpool.tile([128, 128], F32, tag="fo")
                        for j in range(nsub):
                            m = min(128, ws - j * 128)
                            sl = slice(j * 64, j * 64 + 64)
                            nc.vector.tensor_add(fo[0:m, sl], fP[0:m, sl],
                                                 linb[0:m, sl])
                            base = t * cfg.S + w * WIN + j * 128
                            nc.sync.dma_start(out_d[base:base + m, :], fo[0:m, sl])

        stage = getattr(cfg, "stage", 2)
        do_layer(0)
        for t in (0, 1):
            if stage < 1:
                break
            nc.gpsimd.collective_compute(
                "AllGather", mybir.AluOpType.bypass,
                replica_groups=[list(range(NC))],
                ins=[z_loc[t].ap().opt()], outs=[z_full[t].ap().opt()])
        if stage >= 1:
            for t in (0, 1):
                nc.sync.dma_start(z_hi[t].ap(), z_full[t][cfg.HALF:2 * cfg.HALF, :])
        if stage >= 2:
            do_layer(1)

        for p in (pfin, ptrf, pagg, zpool, apool, mpool, ipool, gpool, cpool):
            p.release()

    nc.compile()
    return nc


def _make_inputs(cfg, per_core, x_drug, x_dis, Ws, bs, lin_w, lin_b):
    bf = ml_dtypes.bfloat16
    xb = {0: x_drug.astype(bf), 1: x_dis.astype(bf)}
    wts = np.zeros((2, 2, 2, 128, 128), np.float32)
    b1 = np.zeros((2, 128, 256), np.float32)
    b2 = np.zeros((2, 128, 1), np.float32)
    for l in (0, 1):
        for t in (0, 1):
            for ri in (0, 1):
                r = REL_OF_T[t][ri][0]
                wts[l, t, ri] = Ws[l, r]
            bsum = bs[l, REL_OF_T[t][0][0]] + bs[l, REL_OF_T[t][1][0]]
            if l == 0:
                b1[t] = np.tile(bsum[None, :], (128, 2))
            else:
                b2[t] = bsum[:, None].astype(np.float32)
    shared = {
        "wts": wts.astype(bf),
        "linwt": lin_w.T.astype(bf).copy(),
        "bias1": b1,
        "bias2": b2,
        "linb": np.tile(lin_b[None, :], (128, 2)).astype(np.float32),
    }
    for sd, nm in ((0, "d"), (1, "s")):
        for h in (0, 1):
            shared[f"x_{nm}_h{h}"] = np.ascontiguousarray(
                xb[sd][h * cfg.HALF:(h + 1) * cfg.HALF])
    in_maps = []
    for c in range(cfg.NC):
        m = dict(shared)
        m["idx"] = per_core[c]["idx"]
        m["msel"] = per_core[c]["msel"]
        in_maps.append(m)
    return in_maps


def run(cfg, x_drug, x_dis, eis, Ws, bs, lin_w, lin_b, trace=False):
    edge_arrays = {r: (eis[r][0].astype(np.int64), eis[r][1].astype(np.int64))
                   for r in range(4)}
    meta, per_core = _prep_graph(cfg, edge_arrays)
    nc = _build_program(cfg, meta)
    in_maps = _make_inputs(cfg, per_core, x_drug, x_dis, Ws, bs, lin_w, lin_b)
    res = run_bass_kernel_spmd(nc, in_maps, core_ids=list(range(cfg.NC)),
                               trace=trace)
    drug = np.zeros((cfg.N, cfg.OUT), np.float32)
    dis = np.zeros((cfg.N, cfg.OUT), np.float32)
    for c in range(cfg.NC):
        o = res.results[c]["out"]
        drug[c * cfg.S:(c + 1) * cfg.S] = o[:cfg.S]
        dis[c * cfg.S:(c + 1) * cfg.S] = o[cfg.S:]
    return (drug, dis), res


def kernel(x_drug, x_dis, ei_dd, ei_ss, ei_ds, ei_sd, Ws, bs, lin_w, lin_b):
    cfg = Cfg()
    eis = {0: np.asarray(ei_dd), 1: np.asarray(ei_ss),
           2: np.asarray(ei_ds), 3: np.asarray(ei_sd)}
    out, _ = run(cfg, np.asarray(x_drug), np.asarray(x_dis), eis,
                 np.asarray(Ws), np.asarray(bs),
                 np.asarray(lin_w), np.asarray(lin_b))
    return out



# revision 22
# speedup vs baseline: 1.1242x; 1.0540x over previous
"""HeteroGNN (2-layer hetero GCN) Trainium2 kernel, 8-core SPMD.

Strategy: destination-sharded. Each core owns 6250 drug + 6250 dis nodes.
Feature tables (bf16 rows) live in per-core HBM; edge gathers use
dma_gather (custom SWDGE row gather); scatter-add is done as one-hot
"Msel" matmuls accumulating in PSUM (edges chunked 128 at a time, each
chunk's destinations confined to a 32-wide bin so PSUM offsets are
program constants shared by all cores). Layer-1 output slices are
exchanged with two AllGather collectives, then layer 2 + final linear.
All graph preprocessing (degrees, norms, chunking, padding to the
max-over-cores schedule) happens on host in numpy.
"""

import numpy as np
import ml_dtypes

import sys

for _p in ("/opt/trn_rl_repo",):
    if _p not in sys.path:
        sys.path.insert(0, _p)

import concourse.bass as bass
import concourse.mybir as mybir
from concourse import tile
from concourse.bass_utils import run_bass_kernel_spmd

BF16 = mybir.dt.bfloat16
F32 = mybir.dt.float32
I16 = mybir.dt.int16


class Cfg:
    def __init__(self, n=50000, e=800000, ncores=8, win=256, binw=32, group=1):
        self.N = n              # nodes per type
        self.E = e              # edges per relation
        self.NC = ncores
        self.S = n // ncores    # dst nodes per core per type
        self.WIN = win          # dsts per PSUM window
        self.BINW = binw        # dsts per bin (fixed psum offset granularity)
        self.GROUP = group      # windows per gather call
        self.NW = (self.S + win - 1) // win   # windows per type
        self.NG = (self.NW + group - 1) // group
        self.HALF = n // 2      # rows per gather half-table (int16 idx limit)
        assert self.HALF <= 32768
        self.D = 128
        self.OUT = 64

    def win_size(self, w):
        return min(self.WIN, self.S - w * self.WIN)

    def nbins(self, w):
        ws = self.win_size(w)
        return (ws + self.BINW - 1) // self.BINW


# relations per dst type: (reference rel index, src_is_dis)
# drug dst: rel 0 (dd, src drug), rel 3 (sd, src dis)
# dis  dst: rel 1 (ss, src dis),  rel 2 (ds, src drug)
REL_OF_T = {0: [(0, 0), (3, 1)], 1: [(1, 1), (2, 0)]}
SELF_LOOP = {0: True, 1: True, 2: False, 3: False}


def _prep_graph(cfg, edge_arrays):
    """edge_arrays: dict rel_idx -> (row, col) int64 full edge lists.
    Returns (meta, per_core) where meta is the SPMD-uniform schedule and
    per_core[c] = dict(idx=int16 [128, ICOLS], msel=f32 [128, MCOLS])."""
    N, S, WIN, BINW, NC = cfg.N, cfg.S, cfg.WIN, cfg.BINW, cfg.NC

    # chunks[(t, w, r, h, b)][core] = list of (idx128 array, dloc array, norm array)
    group_chunks = {}
    for t in (0, 1):
        for ri, (r, src_dis) in enumerate(REL_OF_T[t]):
            row, col = edge_arrays[r]
            if SELF_LOOP[r]:
                sl = np.arange(N, dtype=np.int64)
                row = np.concatenate([row, sl])
                col = np.concatenate([col, sl])
            deg_s = np.bincount(row, minlength=N).astype(np.float64)
            deg_d = np.bincount(col, minlength=N).astype(np.float64)
            norm = (deg_s[row] ** -0.5 * deg_d[col] ** -0.5).astype(np.float32)
            core = col // S
            d_loc = col % S
            w = d_loc // WIN
            b = (d_loc % WIN) // BINW
            h = row // cfg.HALF
            idx16 = (row % cfg.HALF).astype(np.int16)
            # group key: (core, w, b, h)
            nb_max = (WIN + BINW - 1) // BINW
            key = ((core * cfg.NW + w) * nb_max + b) * 2 + h
            order = np.argsort(key, kind="stable")
            key_s = key[order]
            uk, starts = np.unique(key_s, return_index=True)
            starts = list(starts) + [len(key_s)]
            for gi, k in enumerate(uk):
                sl_ = order[starts[gi]:starts[gi + 1]]
                kk = int(k)
                hh = kk % 2
                kk //= 2
                bb = kk % nb_max
                kk //= nb_max
                ww = kk % cfg.NW
                cc = kk // cfg.NW
                gkey = (t, ww, ri, hh, bb)
                group_chunks.setdefault(gkey, {c: [] for c in range(NC)})
                lst = group_chunks[gkey][cc]
                for s0 in range(0, len(sl_), 128):
                    ee = sl_[s0:s0 + 128]
                    lst.append((idx16[ee], (d_loc[ee] % WIN) % BINW, norm[ee]))

    # C_max per slot key
    cmax = {}
    for gkey, bycore in group_chunks.items():
        cmax[gkey] = max(len(v) for v in bycore.values())

    # Build uniform schedule.
    # calls: per (t, g, ri, h): list of chunk slot keys in order (w asc, b asc, dup)
    calls = []           # (t, g, ri, h, src_dis, idx_col_off, nchunks)
    call_lookup = {}     # (t, g, ri, h) -> call index
    icol = 0
    for t in (0, 1):
        for g in range(cfg.NG):
            for ri in range(2):
                src_dis = REL_OF_T[t][ri][1]
                for h in (0, 1):
                    nch = 0
                    for w in range(g * cfg.GROUP, min((g + 1) * cfg.GROUP, cfg.NW)):
                        for b in range(cfg.nbins(w)):
                            nch += cmax.get((t, w, ri, h, b), 0)
                    call_lookup[(t, g, ri, h)] = len(calls)
                    calls.append(dict(t=t, g=g, ri=ri, h=h, src_dis=src_dis,
                                      icol=icol, nchunks=nch))
                    icol += nch * 8
    ICOLS = max(icol, 8)

    # windows: per (t, w): msel col offset + chunk list
    windows = {}
    mcol = 0
    for t in (0, 1):
        for w in range(cfg.NW):
            wch = []   # (ri, h, j_in_call, psum_off)
            # j_in_call accumulators per (ri, h) for this group
            for ri in range(2):
                for h in (0, 1):
                    j = 0
                    g = w // cfg.GROUP
                    for w2 in range(g * cfg.GROUP, w):
                        for b in range(cfg.nbins(w2)):
                            j += cmax.get((t, w2, ri, h, b), 0)
                    for b in range(cfg.nbins(w)):
                        for d in range(cmax.get((t, w, ri, h, b), 0)):
                            wch.append((ri, h, j, b * BINW))
                            j += 1
            windows[(t, w)] = dict(mcol=mcol, chunks=wch)
            mcol += len(wch) * BINW
    MCOLS = max(mcol, BINW)

    meta = dict(calls=calls, call_lookup=call_lookup, windows=windows,
                ICOLS=ICOLS, MCOLS=MCOLS)

    # Per-core data arrays
    per_core = []
    for c in range(NC):
        idxa = np.zeros((16, ICOLS), np.int16)
        msel = np.zeros((128, MCOLS), np.float32)
        for call in calls:
            t, g, ri, h = call["t"], call["g"], call["ri"], call["h"]
            j = 0
            for w in range(g * cfg.GROUP, min((g + 1) * cfg.GROUP, cfg.NW)):
                for b in range(cfg.nbins(w)):
                    ck = group_chunks.get((t, w, ri, h, b))
                    lst = ck[c] if ck else []
                    for d in range(cmax.get((t, w, ri, h, b), 0)):
                        if d < len(lst):
                            ii, dd, nn = lst[d]
                            s0 = j * 128
                            sl = np.arange(s0, s0 + len(ii))
                            idxa[sl % 16, call["icol"] + sl // 16] = ii
                        j += 1
        # msel fill: walk windows
        for (t, w), wd in windows.items():
            dupc = {}
            for ci, (ri, h, jc, poff) in enumerate(wd["chunks"]):
                b = poff // BINW
                ck = group_chunks.get((t, w, ri, h, b))
                lst = ck[c] if ck else []
                d = dupc.get((ri, h, poff), 0)
                dupc[(ri, h, poff)] = d + 1
                if d < len(lst):
                    ii, dd, nn = lst[d]
                    m0 = wd["mcol"] + ci * BINW
                    msel[np.arange(len(ii)), m0 + dd] = nn
        idx_full = np.tile(idxa, (8, 1))
        per_core.append(dict(idx=idx_full,
                             msel=msel.astype(ml_dtypes.bfloat16)))
    return meta, per_core


def _build_program(cfg, meta):
    """Build the SPMD Bass program (same for all cores)."""
    from concourse import bacc

    NC, WIN, BINW = cfg.NC, cfg.WIN, cfg.BINW
    GSTEP = 8   # 1024 idxs per dma_gather: >1024 is over the Q7 scratch cap
    NQ = 4      # ucode MAX_SWDGE_QUEUES; rings process entries serially, so
                # spreading instructions over all 4 rings quadruples rate
    nc = bacc.Bacc("TRN2", target_bir_lowering=False, debug=False,
                   num_devices=NC, num_swdge_queues=NQ)

    # I/O
    xt = {}
    for sd, nm in ((0, "d"), (1, "s")):
        for h in (0, 1):
            xt[(sd, h)] = nc.dram_tensor(
                f"x_{nm}_h{h}", [cfg.HALF, 128], BF16, kind="ExternalInput")
    idx_d = nc.dram_tensor("idx", [128, meta["ICOLS"]], I16, kind="ExternalInput")
    msel_d = nc.dram_tensor("msel", [128, meta["MCOLS"]], BF16, kind="ExternalInput")
    wts_d = nc.dram_tensor("wts", [2, 2, 2, 128, 128], BF16, kind="ExternalInput")
    linwt_d = nc.dram_tensor("linwt", [128, cfg.OUT], BF16, kind="ExternalInput")
    bias1_d = nc.dram_tensor("bias1", [2, 128, 2 * 128], F32, kind="ExternalInput")
    bias2_d = nc.dram_tensor("bias2", [2, 128, 1], F32, kind="ExternalInput")
    linb_d = nc.dram_tensor("linb", [128, 2 * cfg.OUT], F32, kind="ExternalInput")
    out_d = nc.dram_tensor("out", [2 * cfg.S, cfg.OUT], F32, kind="ExternalOutput")

    z_loc = [nc.dram_tensor(f"z_loc{t}", [cfg.S, 128], BF16) for t in (0, 1)]
    z_full = [nc.dram_tensor(f"z_full{t}", [cfg.N, 128], BF16,
                             addr_space="Shared") for t in (0, 1)]
    z_hi = [nc.dram_tensor(f"z_hi{t}", [cfg.HALF, 128], BF16) for t in (0, 1)]

    calls, windows = meta["calls"], meta["windows"]
    call_lookup = meta["call_lookup"]

    gctr = [0]

    with tile.TileContext(nc) as tc:
        cpool = tc.alloc_tile_pool(name="const", bufs=1)
        gpool = tc.alloc_tile_pool(name="gather", bufs=4)
        ipool = tc.alloc_tile_pool(name="idx", bufs=4)
        mpool = tc.alloc_tile_pool(name="msel", bufs=2)
        apool = tc.alloc_tile_pool(name="aggs", bufs=2)
        zpool = tc.alloc_tile_pool(name="z", bufs=2)
        pagg = tc.alloc_tile_pool(name="pagg", bufs=2, space="PSUM")
        ptrf = tc.alloc_tile_pool(name="ptrf", bufs=2, space="PSUM")
        pfin = tc.alloc_tile_pool(name="pfin", bufs=2, space="PSUM")

        # constants to SBUF
        wsb = {}
        for l in (0, 1):
            for t in (0, 1):
                for ri in (0, 1):
                    w = cpool.tile([128, 128], BF16, tag=f"w{l}{t}{ri}", name=f"w_{l}{t}{ri}")
                    nc.sync.dma_start(w[:], wts_d[l, t, ri])
                    wsb[(l, t, ri)] = w
        linwt = cpool.tile([128, cfg.OUT], BF16, tag="linwt")
        nc.sync.dma_start(linwt[:], linwt_d[:])
        b1 = {}
        b2 = {}
        for t in (0, 1):
            b1[t] = cpool.tile([128, 256], F32, tag=f"b1{t}", name=f"b1_{t}")
            nc.sync.dma_start(b1[t][:], bias1_d[t])
            b2[t] = cpool.tile([128, 1], F32, tag=f"b2{t}", name=f"b2_{t}")
            nc.sync.dma_start(b2[t][:], bias2_d[t])
        linb = cpool.tile([128, 2 * cfg.OUT], F32, tag="linb")
        nc.sync.dma_start(linb[:], linb_d[:])

        def gather_tables(l, sd):
            if l == 0:
                return [xt[(sd, 0)].ap(), xt[(sd, 1)].ap()]
            return [z_full[sd][0:cfg.HALF, :], z_hi[sd].ap()]

        def do_layer(l):
            gtiles = {}
            for t in (0, 1):
                for w in range(cfg.NW):
                    ws = cfg.win_size(w)
                    g = w // cfg.GROUP
                    if w % cfg.GROUP == 0:
                        for ri in range(2):
                            for h in (0, 1):
                                call = calls[call_lookup[(t, g, ri, h)]]
                                C = call["nchunks"]
                                if C == 0:
                                    gtiles[(ri, h)] = None
                                    continue
                                it = ipool.tile([128, C * 8], I16, tag=f"i{ri}{h}", name=f"it{ri}{h}")
                                nc.sync.dma_start(
                                    it[:], idx_d[:, call["icol"]:call["icol"] + C * 8])
                                gt = gpool.tile([128, C, 128], BF16, tag=f"g{ri}{h}", name=f"gt{ri}{h}")
                                src = gather_tables(l, call["src_dis"])[h]
                                for j0 in range(0, C, GSTEP):
                                    cs = min(GSTEP, C - j0)
                                    nc.gpsimd.dma_gather(
                                        gt[:, j0:j0 + cs, :], src,
                                        it[:, j0 * 8:(j0 + cs) * 8],
                                        cs * 128, cs * 128, 128,
                                        queue_num=gctr[0] % NQ)
                                    gctr[0] += 1
                                gtiles[(ri, h)] = gt
                    wd = windows[(t, w)]
                    nch = len(wd["chunks"])
                    if nch:
                        mt = mpool.tile([128, nch * BINW], BF16, tag="m", name="mt")
                        nc.sync.dma_start(
                            mt[:], msel_d[:, wd["mcol"]:wd["mcol"] + nch * BINW])
                    aggP = [pagg.tile([128, 256], F32, tag=f"agg{r}", name=f"aggP{r}") for r in (0, 1)]
                    nc.vector.memset(aggP[0][:], 0.0)
                    nc.vector.memset(aggP[1][:], 0.0)
                    last_of_r = {}
                    for ci, (ri, h, j, poff) in enumerate(wd["chunks"]):
                        last_of_r[ri] = ci
                    for ci, (ri, h, j, poff) in enumerate(wd["chunks"]):
                        gt = gtiles[(ri, h)]
                        nc.tensor.matmul(
                            aggP[ri][:, poff:poff + BINW],
                            gt[:, j, :],
                            mt[:, ci * BINW:(ci + 1) * BINW],
                            start=False, stop=(last_of_r[ri] == ci),
                            skip_group_check=True)
                    aggS = []
                    for r in (0, 1):
                        a = apool.tile([128, 256], BF16, tag=f"as{r}", name=f"aggS{r}")
                        nc.vector.tensor_copy(a[:, 0:ws], aggP[r][:, 0:ws])
                        aggS.append(a)
                    if l == 0:
                        outP = ptrf.tile([128, 256], F32, tag="tp")
                        nsub = (ws + 127) // 128
                        for j in range(nsub):
                            m = min(128, ws - j * 128)
                            nc.tensor.matmul(
                                outP[0:m, j * 128:j * 128 + 128],
                                aggS[0][:, j * 128:j * 128 + m],
                                wsb[(l, t, 0)][:], start=True, stop=False)
                            nc.tensor.matmul(
                                outP[0:m, j * 128:j * 128 + 128],
                                aggS[1][:, j * 128:j * 128 + m],
                                wsb[(l, t, 1)][:], start=False, stop=True)
                        tmp = zpool.tile([128, 256], F32, tag="tmp")
                        zsb = zpool.tile([128, 256], BF16, tag="zsb")
                        for j in range(nsub):
                            m = min(128, ws - j * 128)
                            sl = slice(j * 128, j * 128 + 128)
                            nc.vector.tensor_add(tmp[0:m, sl], outP[0:m, sl],
                                                 b1[t][0:m, sl])
                            nc.scalar.activation(
                                zsb[0:m, sl], tmp[0:m, sl],
                                mybir.ActivationFunctionType.Relu)
                            base = w * WIN + j * 128
                            nc.sync.dma_start(
                                z_loc[t][base:base + m, :], zsb[0:m, sl])
                    else:
                        z2P = ptrf.tile([128, 256], F32, tag="tp")
                        nc.tensor.matmul(z2P[:, 0:ws], wsb[(l, t, 0)][:],
                                         aggS[0][:, 0:ws], start=True, stop=False)
                        nc.tensor.matmul(z2P[:, 0:ws], wsb[(l, t, 1)][:],
                                         aggS[1][:, 0:ws], start=False, stop=True)
                        z2T = zpool.tile([128, 256], BF16, tag="z2t")
                        nc.scalar.activation(z2T[:, 0:ws], z2P[:, 0:ws],
                                             mybir.ActivationFunctionType.Relu,
                                             bias=b2[t][:])
                        fP = pfin.tile([128, 128], F32, tag="fp")
                        nsub = (ws + 127) // 128
                        for j in range(nsub):
                            m = min(128, ws - j * 128)
                            nc.tensor.matmul(
                                fP[0:m, j * 64:j * 64 + 64],
                                z2T[:, j * 128:j * 128 + m],
                                linwt[:], start=True, stop=True)
                        fo = zpool.tile([128, 128], F32, tag="fo")
                        for j in range(nsub):
                            m = min(128, ws - j * 128)
                            sl = slice(j * 64, j * 64 + 64)
                            nc.vector.tensor_add(fo[0:m, sl], fP[0:m, sl],
                                                 linb[0:m, sl])
                            base = t * cfg.S + w * WIN + j * 128
                            nc.sync.dma_start(out_d[base:base + m, :], fo[0:m, sl])

        stage = getattr(cfg, "stage", 2)
        do_layer(0)
        for t in (0, 1):
            if stage < 1:
                break
            nc.gpsimd.collective_compute(
                "AllGather", mybir.AluOpType.bypass,
                replica_groups=[list(range(NC))],
                ins=[z_loc[t].ap().opt()], outs=[z_full[t].ap().opt()])
        if stage >= 1:
            tc.strict_bb_all_engine_barrier()
            for t in (0, 1):
                nc.sync.dma_start(z_hi[t].ap(), z_full[t][cfg.HALF:2 * cfg.HALF, :])
        if stage >= 2:
            do_layer(1)

        for p in (pfin, ptrf, pagg, zpool, apool, mpool, ipool, gpool, cpool):
            p.release()

    nc.compile()
    return nc


def _make_inputs(cfg, per_core, x_drug, x_dis, Ws, bs, lin_w, lin_b):
    bf = ml_dtypes.bfloat16
    xb = {0: x_drug.astype(bf), 1: x_dis.astype(bf)}
    wts = np.zeros((2, 2, 2, 128, 128), np.float32)
    b1 = np.zeros((2, 128, 256), np.float32)
    b2 = np.zeros((2, 128, 1), np.float32)
    for l in (0, 1):
        for t in (0, 1):
            for ri in (0, 1):
                r = REL_OF_T[t][ri][0]
                wts[l, t, ri] = Ws[l, r]
            bsum = bs[l, REL_OF_T[t][0][0]] + bs[l, REL_OF_T[t][1][0]]
            if l == 0:
                b1[t] = np.tile(bsum[None, :], (128, 2))
            else:
                b2[t] = bsum[:, None].astype(np.float32)
    shared = {
        "wts": wts.astype(bf),
        "linwt": lin_w.T.astype(bf).copy(),
        "bias1": b1,
        "bias2": b2,
        "linb": np.tile(lin_b[None, :], (128, 2)).astype(np.float32),
    }
    for sd, nm in ((0, "d"), (1, "s")):
        for h in (0, 1):
            shared[f"x_{nm}_h{h}"] = np.ascontiguousarray(
                xb[sd][h * cfg.HALF:(h + 1) * cfg.HALF])
    in_maps = []
    for c in range(cfg.NC):
        m = dict(shared)
        m["idx"] = per_core[c]["idx"]
        m["msel"] = per_core[c]["msel"]
        in_maps.append(m)
    return in_maps


def run(cfg, x_drug, x_dis, eis, Ws, bs, lin_w, lin_b, trace=False):
    edge_arrays = {r: (eis[r][0].astype(np.int64), eis[r][1].astype(np.int64))
                   for r in range(4)}
    meta, per_core = _prep_graph(cfg, edge_arrays)
    nc = _build_program(cfg, meta)
    in_maps = _make_inputs(cfg, per_core, x_drug, x_dis, Ws, bs, lin_w, lin_b)
    res = run_bass_kernel_spmd(nc, in_maps, core_ids=list(range(cfg.NC)),
                               trace=trace)
    drug = np.zeros((cfg.N, cfg.OUT), np.float32)
    dis = np.zeros((cfg.N, cfg.OUT), np.float32)
    for c in range(cfg.NC):
        o = res.results[c]["out"]
        drug[c * cfg.S:(c + 1) * cfg.S] = o[:cfg.S]
        dis[c * cfg.S:(c + 1) * cfg.S] = o[cfg.S:]
    return (drug, dis), res


def kernel(x_drug, x_dis, ei_dd, ei_ss, ei_ds, ei_sd, Ws, bs, lin_w, lin_b):
    cfg = Cfg()
    eis = {0: np.asarray(ei_dd), 1: np.asarray(ei_ss),
           2: np.asarray(ei_ds), 3: np.asarray(ei_sd)}
    out, _ = run(cfg, np.asarray(x_drug), np.asarray(x_dis), eis,
                 np.asarray(Ws), np.asarray(bs),
                 np.asarray(lin_w), np.asarray(lin_b))
    return out



# revision 23
# speedup vs baseline: 1.2835x; 1.1417x over previous
"""HeteroGNN (2-layer hetero GCN) Trainium2 kernel, 8-core SPMD.

Strategy: destination-sharded. Each core owns 6250 drug + 6250 dis nodes.
Feature tables (bf16 rows) live in per-core HBM; edge gathers use
dma_gather (custom SWDGE row gather); scatter-add is done as one-hot
"Msel" matmuls accumulating in PSUM (edges chunked 128 at a time, each
chunk's destinations confined to a 32-wide bin so PSUM offsets are
program constants shared by all cores). Layer-1 output slices are
exchanged with two AllGather collectives, then layer 2 + final linear.
All graph preprocessing (degrees, norms, chunking, padding to the
max-over-cores schedule) happens on host in numpy.
"""

import numpy as np
import ml_dtypes

import sys

for _p in ("/opt/trn_rl_repo",):
    if _p not in sys.path:
        sys.path.insert(0, _p)

import concourse.bass as bass
import concourse.mybir as mybir
from concourse import tile
from concourse.bass_utils import run_bass_kernel_spmd

BF16 = mybir.dt.bfloat16
F32 = mybir.dt.float32
I16 = mybir.dt.int16


class Cfg:
    def __init__(self, n=50000, e=800000, ncores=8, win=256, binw=128, group=1):
        self.N = n              # nodes per type
        self.E = e              # edges per relation
        self.NC = ncores
        self.S = n // ncores    # dst nodes per core per type
        self.WIN = win          # dsts per PSUM window
        self.BINW = binw        # dsts per bin (fixed psum offset granularity)
        self.GROUP = group      # windows per gather call
        self.NW = (self.S + win - 1) // win   # windows per type
        self.NG = (self.NW + group - 1) // group
        self.HALF = n // 2      # rows per gather half-table (int16 idx limit)
        assert self.HALF <= 32768
        self.D = 128
        self.OUT = 64

    def win_size(self, w):
        return min(self.WIN, self.S - w * self.WIN)

    def nbins(self, w):
        ws = self.win_size(w)
        return (ws + self.BINW - 1) // self.BINW


# relations per dst type: (reference rel index, src_is_dis)
# drug dst: rel 0 (dd, src drug), rel 3 (sd, src dis)
# dis  dst: rel 1 (ss, src dis),  rel 2 (ds, src drug)
REL_OF_T = {0: [(0, 0), (3, 1)], 1: [(1, 1), (2, 0)]}
SELF_LOOP = {0: True, 1: True, 2: False, 3: False}


def _prep_graph(cfg, edge_arrays):
    """edge_arrays: dict rel_idx -> (row, col) int64 full edge lists.
    Returns (meta, per_core) where meta is the SPMD-uniform schedule and
    per_core[c] = dict(idx=int16 [128, ICOLS], msel=f32 [128, MCOLS])."""
    N, S, WIN, BINW, NC = cfg.N, cfg.S, cfg.WIN, cfg.BINW, cfg.NC

    # chunks[(t, w, r, h, b)][core] = list of (idx128 array, dloc array, norm array)
    group_chunks = {}
    for t in (0, 1):
        for ri, (r, src_dis) in enumerate(REL_OF_T[t]):
            row, col = edge_arrays[r]
            if SELF_LOOP[r]:
                sl = np.arange(N, dtype=np.int64)
                row = np.concatenate([row, sl])
                col = np.concatenate([col, sl])
            deg_s = np.bincount(row, minlength=N).astype(np.float64)
            deg_d = np.bincount(col, minlength=N).astype(np.float64)
            norm = (deg_s[row] ** -0.5 * deg_d[col] ** -0.5).astype(np.float32)
            core = col // S
            d_loc = col % S
            w = d_loc // WIN
            b = (d_loc % WIN) // BINW
            h = row // cfg.HALF
            idx16 = (row % cfg.HALF).astype(np.int16)
            # group key: (core, w, b, h)
            nb_max = (WIN + BINW - 1) // BINW
            key = ((core * cfg.NW + w) * nb_max + b) * 2 + h
            order = np.argsort(key, kind="stable")
            key_s = key[order]
            uk, starts = np.unique(key_s, return_index=True)
            starts = list(starts) + [len(key_s)]
            for gi, k in enumerate(uk):
                sl_ = order[starts[gi]:starts[gi + 1]]
                kk = int(k)
                hh = kk % 2
                kk //= 2
                bb = kk % nb_max
                kk //= nb_max
                ww = kk % cfg.NW
                cc = kk // cfg.NW
                gkey = (t, ww, ri, hh, bb)
                group_chunks.setdefault(gkey, {c: [] for c in range(NC)})
                lst = group_chunks[gkey][cc]
                for s0 in range(0, len(sl_), 128):
                    ee = sl_[s0:s0 + 128]
                    lst.append((idx16[ee], (d_loc[ee] % WIN) % BINW, norm[ee]))

    # C_max per slot key
    cmax = {}
    for gkey, bycore in group_chunks.items():
        cmax[gkey] = max(len(v) for v in bycore.values())

    # Build uniform schedule.
    # calls: per (t, g, ri, h): list of chunk slot keys in order (w asc, b asc, dup)
    calls = []           # (t, g, ri, h, src_dis, idx_col_off, nchunks)
    call_lookup = {}     # (t, g, ri, h) -> call index
    icol = 0
    for t in (0, 1):
        for g in range(cfg.NG):
            for ri in range(2):
                src_dis = REL_OF_T[t][ri][1]
                for h in (0, 1):
                    nch = 0
                    for w in range(g * cfg.GROUP, min((g + 1) * cfg.GROUP, cfg.NW)):
                        for b in range(cfg.nbins(w)):
                            nch += cmax.get((t, w, ri, h, b), 0)
                    call_lookup[(t, g, ri, h)] = len(calls)
                    calls.append(dict(t=t, g=g, ri=ri, h=h, src_dis=src_dis,
                                      icol=icol, nchunks=nch))
                    icol += nch * 8
    ICOLS = max(icol, 8)

    # windows: per (t, w): msel col offset + chunk list
    windows = {}
    mcol = 0
    for t in (0, 1):
        for w in range(cfg.NW):
            wch = []   # (ri, h, j_in_call, psum_off)
            # j_in_call accumulators per (ri, h) for this group
            for ri in range(2):
                for h in (0, 1):
                    j = 0
                    g = w // cfg.GROUP
                    for w2 in range(g * cfg.GROUP, w):
                        for b in range(cfg.nbins(w2)):
                            j += cmax.get((t, w2, ri, h, b), 0)
                    for b in range(cfg.nbins(w)):
                        for d in range(cmax.get((t, w, ri, h, b), 0)):
                            wch.append((ri, h, j, b * BINW))
                            j += 1
            windows[(t, w)] = dict(mcol=mcol, chunks=wch)
            mcol += len(wch) * BINW
    MCOLS = max(mcol, BINW)

    meta = dict(calls=calls, call_lookup=call_lookup, windows=windows,
                ICOLS=ICOLS, MCOLS=MCOLS)

    # Per-core data arrays
    per_core = []
    for c in range(NC):
        idxa = np.zeros((16, ICOLS), np.int16)
        msel = np.zeros((128, MCOLS), np.float32)
        for call in calls:
            t, g, ri, h = call["t"], call["g"], call["ri"], call["h"]
            j = 0
            for w in range(g * cfg.GROUP, min((g + 1) * cfg.GROUP, cfg.NW)):
                for b in range(cfg.nbins(w)):
                    ck = group_chunks.get((t, w, ri, h, b))
                    lst = ck[c] if ck else []
                    for d in range(cmax.get((t, w, ri, h, b), 0)):
                        if d < len(lst):
                            ii, dd, nn = lst[d]
                            s0 = j * 128
                            sl = np.arange(s0, s0 + len(ii))
                            idxa[sl % 16, call["icol"] + sl // 16] = ii
                        j += 1
        # msel fill: walk windows
        for (t, w), wd in windows.items():
            dupc = {}
            for ci, (ri, h, jc, poff) in enumerate(wd["chunks"]):
                b = poff // BINW
                ck = group_chunks.get((t, w, ri, h, b))
                lst = ck[c] if ck else []
                d = dupc.get((ri, h, poff), 0)
                dupc[(ri, h, poff)] = d + 1
                if d < len(lst):
                    ii, dd, nn = lst[d]
                    m0 = wd["mcol"] + ci * BINW
                    msel[np.arange(len(ii)), m0 + dd] = nn
        idx_full = np.tile(idxa, (8, 1))
        per_core.append(dict(idx=idx_full,
                             msel=msel.astype(ml_dtypes.bfloat16)))
    return meta, per_core


def _build_program(cfg, meta):
    """Build the SPMD Bass program (same for all cores)."""
    from concourse import bacc

    NC, WIN, BINW = cfg.NC, cfg.WIN, cfg.BINW
    GSTEP = 8   # 1024 idxs per dma_gather: >1024 is over the Q7 scratch cap
    NQ = 4      # ucode MAX_SWDGE_QUEUES; rings process entries serially, so
                # spreading instructions over all 4 rings quadruples rate
    nc = bacc.Bacc("TRN2", target_bir_lowering=False, debug=False,
                   num_devices=NC, num_swdge_queues=NQ)

    # I/O
    xt = {}
    for sd, nm in ((0, "d"), (1, "s")):
        for h in (0, 1):
            xt[(sd, h)] = nc.dram_tensor(
                f"x_{nm}_h{h}", [cfg.HALF, 128], BF16, kind="ExternalInput")
    idx_d = nc.dram_tensor("idx", [128, meta["ICOLS"]], I16, kind="ExternalInput")
    msel_d = nc.dram_tensor("msel", [128, meta["MCOLS"]], BF16, kind="ExternalInput")
    wts_d = nc.dram_tensor("wts", [2, 2, 2, 128, 128], BF16, kind="ExternalInput")
    linwt_d = nc.dram_tensor("linwt", [128, cfg.OUT], BF16, kind="ExternalInput")
    bias1_d = nc.dram_tensor("bias1", [2, 128, 2 * 128], F32, kind="ExternalInput")
    bias2_d = nc.dram_tensor("bias2", [2, 128, 1], F32, kind="ExternalInput")
    linb_d = nc.dram_tensor("linb", [128, 2 * cfg.OUT], F32, kind="ExternalInput")
    out_d = nc.dram_tensor("out", [2 * cfg.S, cfg.OUT], F32, kind="ExternalOutput")

    z_loc = [nc.dram_tensor(f"z_loc{t}", [cfg.S, 128], BF16) for t in (0, 1)]
    z_full = [nc.dram_tensor(f"z_full{t}", [cfg.N, 128], BF16,
                             addr_space="Shared") for t in (0, 1)]
    z_hi = [nc.dram_tensor(f"z_hi{t}", [cfg.HALF, 128], BF16) for t in (0, 1)]

    calls, windows = meta["calls"], meta["windows"]
    call_lookup = meta["call_lookup"]

    gctr = [0]

    with tile.TileContext(nc) as tc:
        cpool = tc.alloc_tile_pool(name="const", bufs=1)
        gpool = tc.alloc_tile_pool(name="gather", bufs=4)
        ipool = tc.alloc_tile_pool(name="idx", bufs=8)
        mpool = tc.alloc_tile_pool(name="msel", bufs=2)
        apool = tc.alloc_tile_pool(name="aggs", bufs=2)
        zpool = tc.alloc_tile_pool(name="z", bufs=2)
        pagg = tc.alloc_tile_pool(name="pagg", bufs=2, space="PSUM")
        ptrf = tc.alloc_tile_pool(name="ptrf", bufs=2, space="PSUM")
        pfin = tc.alloc_tile_pool(name="pfin", bufs=2, space="PSUM")

        # constants to SBUF
        wsb = {}
        for l in (0, 1):
            for t in (0, 1):
                for ri in (0, 1):
                    w = cpool.tile([128, 128], BF16, tag=f"w{l}{t}{ri}", name=f"w_{l}{t}{ri}")
                    nc.sync.dma_start(w[:], wts_d[l, t, ri])
                    wsb[(l, t, ri)] = w
        linwt = cpool.tile([128, cfg.OUT], BF16, tag="linwt")
        nc.sync.dma_start(linwt[:], linwt_d[:])
        b1 = {}
        b2 = {}
        for t in (0, 1):
            b1[t] = cpool.tile([128, 256], F32, tag=f"b1{t}", name=f"b1_{t}")
            nc.sync.dma_start(b1[t][:], bias1_d[t])
            b2[t] = cpool.tile([128, 1], F32, tag=f"b2{t}", name=f"b2_{t}")
            nc.sync.dma_start(b2[t][:], bias2_d[t])
        linb = cpool.tile([128, 2 * cfg.OUT], F32, tag="linb")
        nc.sync.dma_start(linb[:], linb_d[:])

        def gather_tables(l, sd):
            if l == 0:
                return [xt[(sd, 0)].ap(), xt[(sd, 1)].ap()]
            return [z_full[sd][0:cfg.HALF, :], z_hi[sd].ap()]

        def do_layer(l):
            gtiles = {}
            for t in (0, 1):
                for w in range(cfg.NW):
                    ws = cfg.win_size(w)
                    g = w // cfg.GROUP
                    if w % cfg.GROUP == 0:
                        for ri in range(2):
                            for h in (0, 1):
                                call = calls[call_lookup[(t, g, ri, h)]]
                                C = call["nchunks"]
                                if C == 0:
                                    gtiles[(ri, h)] = None
                                    continue
                                it = ipool.tile([128, C * 8], I16, tag=f"i{ri}{h}", name=f"it{ri}{h}")
                                nc.sync.dma_start(
                                    it[:], idx_d[:, call["icol"]:call["icol"] + C * 8])
                                gt = gpool.tile([128, C, 128], BF16, tag=f"g{ri}{h}", name=f"gt{ri}{h}")
                                src = gather_tables(l, call["src_dis"])[h]
                                for j0 in range(0, C, GSTEP):
                                    cs = min(GSTEP, C - j0)
                                    nc.gpsimd.dma_gather(
                                        gt[:, j0:j0 + cs, :], src,
                                        it[:, j0 * 8:(j0 + cs) * 8],
                                        cs * 128, cs * 128, 128,
                                        queue_num=gctr[0] % NQ)
                                    gctr[0] += 1
                                gtiles[(ri, h)] = gt
                    wd = windows[(t, w)]
                    nch = len(wd["chunks"])
                    if nch:
                        mt = mpool.tile([128, nch * BINW], BF16, tag="m", name="mt")
                        for c0 in range(0, nch * BINW, 512):
                            c1 = min(c0 + 512, nch * BINW)
                            nc.sync.dma_start(
                                mt[:, c0:c1],
                                msel_d[:, wd["mcol"] + c0:wd["mcol"] + c1])
                    aggP = [pagg.tile([128, 256], F32, tag=f"agg{r}", name=f"aggP{r}") for r in (0, 1)]
                    nc.vector.memset(aggP[0][:], 0.0)
                    nc.vector.memset(aggP[1][:], 0.0)
                    last_of_r = {}
                    for ci, (ri, h, j, poff) in enumerate(wd["chunks"]):
                        last_of_r[ri] = ci
                    for ci, (ri, h, j, poff) in enumerate(wd["chunks"]):
                        gt = gtiles[(ri, h)]
                        nc.tensor.matmul(
                            aggP[ri][:, poff:poff + BINW],
                            gt[:, j, :],
                            mt[:, ci * BINW:(ci + 1) * BINW],
                            start=False, stop=(last_of_r[ri] == ci),
                            skip_group_check=True)
                    aggS = []
                    for r in (0, 1):
                        a = apool.tile([128, 256], BF16, tag=f"as{r}", name=f"aggS{r}")
                        nc.vector.tensor_copy(a[:, 0:ws], aggP[r][:, 0:ws])
                        aggS.append(a)
                    if l == 0:
                        outP = ptrf.tile([128, 256], F32, tag="tp")
                        nsub = (ws + 127) // 128
                        for j in range(nsub):
                            m = min(128, ws - j * 128)
                            nc.tensor.matmul(
                                outP[0:m, j * 128:j * 128 + 128],
                                aggS[0][:, j * 128:j * 128 + m],
                                wsb[(l, t, 0)][:], start=True, stop=False)
                            nc.tensor.matmul(
                                outP[0:m, j * 128:j * 128 + 128],
                                aggS[1][:, j * 128:j * 128 + m],
                                wsb[(l, t, 1)][:], start=False, stop=True)
                        tmp = zpool.tile([128, 256], F32, tag="tmp")
                        zsb = zpool.tile([128, 256], BF16, tag="zsb")
                        for j in range(nsub):
                            m = min(128, ws - j * 128)
                            sl = slice(j * 128, j * 128 + 128)
                            nc.vector.tensor_add(tmp[0:m, sl], outP[0:m, sl],
                                                 b1[t][0:m, sl])
                            nc.scalar.activation(
                                zsb[0:m, sl], tmp[0:m, sl],
                                mybir.ActivationFunctionType.Relu)
                            base = w * WIN + j * 128
                            nc.sync.dma_start(
                                z_loc[t][base:base + m, :], zsb[0:m, sl])
                    else:
                        z2P = ptrf.tile([128, 256], F32, tag="tp")
                        nc.tensor.matmul(z2P[:, 0:ws], wsb[(l, t, 0)][:],
                                         aggS[0][:, 0:ws], start=True, stop=False)
                        nc.tensor.matmul(z2P[:, 0:ws], wsb[(l, t, 1)][:],
                                         aggS[1][:, 0:ws], start=False, stop=True)
                        z2T = zpool.tile([128, 256], BF16, tag="z2t")
                        nc.scalar.activation(z2T[:, 0:ws], z2P[:, 0:ws],
                                             mybir.ActivationFunctionType.Relu,
                                             bias=b2[t][:])
                        fP = pfin.tile([128, 128], F32, tag="fp")
                        nsub = (ws + 127) // 128
                        for j in range(nsub):
                            m = min(128, ws - j * 128)
                            nc.tensor.matmul(
                                fP[0:m, j * 64:j * 64 + 64],
                                z2T[:, j * 128:j * 128 + m],
                                linwt[:], start=True, stop=True)
                        fo = zpool.tile([128, 128], F32, tag="fo")
                        for j in range(nsub):
                            m = min(128, ws - j * 128)
                            sl = slice(j * 64, j * 64 + 64)
                            nc.vector.tensor_add(fo[0:m, sl], fP[0:m, sl],
                                                 linb[0:m, sl])
                            base = t * cfg.S + w * WIN + j * 128
                            nc.sync.dma_start(out_d[base:base + m, :], fo[0:m, sl])

        stage = getattr(cfg, "stage", 2)
        do_layer(0)
        for t in (0, 1):
            if stage < 1:
                break
            nc.gpsimd.collective_compute(
                "AllGather", mybir.AluOpType.bypass,
                replica_groups=[list(range(NC))],
                ins=[z_loc[t].ap().opt()], outs=[z_full[t].ap().opt()])
        if stage >= 1:
            tc.strict_bb_all_engine_barrier()
            for t in (0, 1):
                nc.sync.dma_start(z_hi[t].ap(), z_full[t][cfg.HALF:2 * cfg.HALF, :])
        if stage >= 2:
            do_layer(1)

        for p in (pfin, ptrf, pagg, zpool, apool, mpool, ipool, gpool, cpool):
            p.release()

    nc.compile()
    return nc


def _make_inputs(cfg, per_core, x_drug, x_dis, Ws, bs, lin_w, lin_b):
    bf = ml_dtypes.bfloat16
    xb = {0: x_drug.astype(bf), 1: x_dis.astype(bf)}
    wts = np.zeros((2, 2, 2, 128, 128), np.float32)
    b1 = np.zeros((2, 128, 256), np.float32)
    b2 = np.zeros((2, 128, 1), np.float32)
    for l in (0, 1):
        for t in (0, 1):
            for ri in (0, 1):
                r = REL_OF_T[t][ri][0]
                wts[l, t, ri] = Ws[l, r]
            bsum = bs[l, REL_OF_T[t][0][0]] + bs[l, REL_OF_T[t][1][0]]
            if l == 0:
                b1[t] = np.tile(bsum[None, :], (128, 2))
            else:
                b2[t] = bsum[:, None].astype(np.float32)
    shared = {
        "wts": wts.astype(bf),
        "linwt": lin_w.T.astype(bf).copy(),
        "bias1": b1,
        "bias2": b2,
        "linb": np.tile(lin_b[None, :], (128, 2)).astype(np.float32),
    }
    for sd, nm in ((0, "d"), (1, "s")):
        for h in (0, 1):
            shared[f"x_{nm}_h{h}"] = np.ascontiguousarray(
                xb[sd][h * cfg.HALF:(h + 1) * cfg.HALF])
    in_maps = []
    for c in range(cfg.NC):
        m = dict(shared)
        m["idx"] = per_core[c]["idx"]
        m["msel"] = per_core[c]["msel"]
        in_maps.append(m)
    return in_maps


def run(cfg, x_drug, x_dis, eis, Ws, bs, lin_w, lin_b, trace=False):
    edge_arrays = {r: (eis[r][0].astype(np.int64), eis[r][1].astype(np.int64))
                   for r in range(4)}
    meta, per_core = _prep_graph(cfg, edge_arrays)
    nc = _build_program(cfg, meta)
    in_maps = _make_inputs(cfg, per_core, x_drug, x_dis, Ws, bs, lin_w, lin_b)
    res = run_bass_kernel_spmd(nc, in_maps, core_ids=list(range(cfg.NC)),
                               trace=trace)
    drug = np.zeros((cfg.N, cfg.OUT), np.float32)
    dis = np.zeros((cfg.N, cfg.OUT), np.float32)
    for c in range(cfg.NC):
        o = res.results[c]["out"]
        drug[c * cfg.S:(c + 1) * cfg.S] = o[:cfg.S]
        dis[c * cfg.S:(c + 1) * cfg.S] = o[cfg.S:]
    return (drug, dis), res


def kernel(x_drug, x_dis, ei_dd, ei_ss, ei_ds, ei_sd, Ws, bs, lin_w, lin_b):
    cfg = Cfg()
    eis = {0: np.asarray(ei_dd), 1: np.asarray(ei_ss),
           2: np.asarray(ei_ds), 3: np.asarray(ei_sd)}
    out, _ = run(cfg, np.asarray(x_drug), np.asarray(x_dis), eis,
                 np.asarray(Ws), np.asarray(bs),
                 np.asarray(lin_w), np.asarray(lin_b))
    return out



# revision 24
# speedup vs baseline: 1.2855x; 1.0015x over previous
"""HeteroGNN (2-layer hetero GCN) Trainium2 kernel, 8-core SPMD.

Strategy: destination-sharded. Each core owns 6250 drug + 6250 dis nodes.
Feature tables (bf16 rows) live in per-core HBM; edge gathers use
dma_gather (custom SWDGE row gather); scatter-add is done as one-hot
"Msel" matmuls accumulating in PSUM (edges chunked 128 at a time, each
chunk's destinations confined to a 32-wide bin so PSUM offsets are
program constants shared by all cores). Layer-1 output slices are
exchanged with two AllGather collectives, then layer 2 + final linear.
All graph preprocessing (degrees, norms, chunking, padding to the
max-over-cores schedule) happens on host in numpy.
"""

import numpy as np
import ml_dtypes

import sys

for _p in ("/opt/trn_rl_repo",):
    if _p not in sys.path:
        sys.path.insert(0, _p)

import concourse.bass as bass
import concourse.mybir as mybir
from concourse import tile
from concourse.bass_utils import run_bass_kernel_spmd

BF16 = mybir.dt.bfloat16
F32 = mybir.dt.float32
I16 = mybir.dt.int16


class Cfg:
    def __init__(self, n=50000, e=800000, ncores=8, win=256, binw=128, group=1):
        self.N = n              # nodes per type
        self.E = e              # edges per relation
        self.NC = ncores
        self.S = n // ncores    # dst nodes per core per type
        self.WIN = win          # dsts per PSUM window
        self.BINW = binw        # dsts per bin (fixed psum offset granularity)
        self.GROUP = group      # windows per gather call
        self.NW = (self.S + win - 1) // win   # windows per type
        self.NG = (self.NW + group - 1) // group
        self.HALF = n // 2      # rows per gather half-table (int16 idx limit)
        assert self.HALF <= 32768
        self.D = 128
        self.OUT = 64

    def win_size(self, w):
        return min(self.WIN, self.S - w * self.WIN)

    def nbins(self, w):
        ws = self.win_size(w)
        return (ws + self.BINW - 1) // self.BINW


# relations per dst type: (reference rel index, src_is_dis)
# drug dst: rel 0 (dd, src drug), rel 3 (sd, src dis)
# dis  dst: rel 1 (ss, src dis),  rel 2 (ds, src drug)
REL_OF_T = {0: [(0, 0), (3, 1)], 1: [(1, 1), (2, 0)]}
SELF_LOOP = {0: True, 1: True, 2: False, 3: False}


def _prep_graph(cfg, edge_arrays):
    """edge_arrays: dict rel_idx -> (row, col) int64 full edge lists.
    Returns (meta, per_core) where meta is the SPMD-uniform schedule and
    per_core[c] = dict(idx=int16 [128, ICOLS], msel=f32 [128, MCOLS])."""
    N, S, WIN, BINW, NC = cfg.N, cfg.S, cfg.WIN, cfg.BINW, cfg.NC

    # chunks[(t, w, r, h, b)][core] = list of (idx128 array, dloc array, norm array)
    group_chunks = {}
    for t in (0, 1):
        for ri, (r, src_dis) in enumerate(REL_OF_T[t]):
            row, col = edge_arrays[r]
            if SELF_LOOP[r]:
                sl = np.arange(N, dtype=np.int64)
                row = np.concatenate([row, sl])
                col = np.concatenate([col, sl])
            deg_s = np.bincount(row, minlength=N).astype(np.float64)
            deg_d = np.bincount(col, minlength=N).astype(np.float64)
            norm = (deg_s[row] ** -0.5 * deg_d[col] ** -0.5).astype(np.float32)
            core = col // S
            d_loc = col % S
            w = d_loc // WIN
            b = (d_loc % WIN) // BINW
            h = row // cfg.HALF
            idx16 = (row % cfg.HALF).astype(np.int16)
            # group key: (core, w, b, h)
            nb_max = (WIN + BINW - 1) // BINW
            key = ((core * cfg.NW + w) * nb_max + b) * 2 + h
            order = np.argsort(key, kind="stable")
            key_s = key[order]
            uk, starts = np.unique(key_s, return_index=True)
            starts = list(starts) + [len(key_s)]
            for gi, k in enumerate(uk):
                sl_ = order[starts[gi]:starts[gi + 1]]
                kk = int(k)
                hh = kk % 2
                kk //= 2
                bb = kk % nb_max
                kk //= nb_max
                ww = kk % cfg.NW
                cc = kk // cfg.NW
                gkey = (t, ww, ri, hh, bb)
                group_chunks.setdefault(gkey, {c: [] for c in range(NC)})
                lst = group_chunks[gkey][cc]
                for s0 in range(0, len(sl_), 128):
                    ee = sl_[s0:s0 + 128]
                    lst.append((idx16[ee], (d_loc[ee] % WIN) % BINW, norm[ee]))

    # C_max per slot key
    cmax = {}
    for gkey, bycore in group_chunks.items():
        cmax[gkey] = max(len(v) for v in bycore.values())

    # Build uniform schedule.
    # calls: per (t, g, ri, h): list of chunk slot keys in order (w asc, b asc, dup)
    calls = []           # (t, g, ri, h, src_dis, idx_col_off, nchunks)
    call_lookup = {}     # (t, g, ri, h) -> call index
    icol = 0
    for t in (0, 1):
        for g in range(cfg.NG):
            for ri in range(2):
                src_dis = REL_OF_T[t][ri][1]
                for h in (0, 1):
                    nch = 0
                    for w in range(g * cfg.GROUP, min((g + 1) * cfg.GROUP, cfg.NW)):
                        for b in range(cfg.nbins(w)):
                            nch += cmax.get((t, w, ri, h, b), 0)
                    call_lookup[(t, g, ri, h)] = len(calls)
                    calls.append(dict(t=t, g=g, ri=ri, h=h, src_dis=src_dis,
                                      icol=icol, nchunks=nch))
                    icol += nch * 8
    ICOLS = max(icol, 8)

    # windows: per (t, w): msel col offset + chunk list
    windows = {}
    mcol = 0
    for t in (0, 1):
        for w in range(cfg.NW):
            wch = []   # (ri, h, j_in_call, psum_off)
            # j_in_call accumulators per (ri, h) for this group
            for ri in range(2):
                for h in (0, 1):
                    j = 0
                    g = w // cfg.GROUP
                    for w2 in range(g * cfg.GROUP, w):
                        for b in range(cfg.nbins(w2)):
                            j += cmax.get((t, w2, ri, h, b), 0)
                    for b in range(cfg.nbins(w)):
                        for d in range(cmax.get((t, w, ri, h, b), 0)):
                            wch.append((ri, h, j, b * BINW))
                            j += 1
            windows[(t, w)] = dict(mcol=mcol, chunks=wch)
            mcol += len(wch) * BINW
    MCOLS = max(mcol, BINW)

    meta = dict(calls=calls, call_lookup=call_lookup, windows=windows,
                ICOLS=ICOLS, MCOLS=MCOLS)

    # Per-core data arrays
    per_core = []
    for c in range(NC):
        idxa = np.zeros((16, ICOLS), np.int16)
        msel = np.zeros((128, MCOLS), np.float32)
        for call in calls:
            t, g, ri, h = call["t"], call["g"], call["ri"], call["h"]
            j = 0
            for w in range(g * cfg.GROUP, min((g + 1) * cfg.GROUP, cfg.NW)):
                for b in range(cfg.nbins(w)):
                    ck = group_chunks.get((t, w, ri, h, b))
                    lst = ck[c] if ck else []
                    for d in range(cmax.get((t, w, ri, h, b), 0)):
                        if d < len(lst):
                            ii, dd, nn = lst[d]
                            s0 = j * 128
                            sl = np.arange(s0, s0 + len(ii))
                            idxa[sl % 16, call["icol"] + sl // 16] = ii
                        j += 1
        # msel fill: walk windows
        for (t, w), wd in windows.items():
            dupc = {}
            for ci, (ri, h, jc, poff) in enumerate(wd["chunks"]):
                b = poff // BINW
                ck = group_chunks.get((t, w, ri, h, b))
                lst = ck[c] if ck else []
                d = dupc.get((ri, h, poff), 0)
                dupc[(ri, h, poff)] = d + 1
                if d < len(lst):
                    ii, dd, nn = lst[d]
                    m0 = wd["mcol"] + ci * BINW
                    msel[np.arange(len(ii)), m0 + dd] = nn
        idx_full = np.tile(idxa, (8, 1))
        per_core.append(dict(idx=idx_full,
                             msel=msel.astype(ml_dtypes.bfloat16)))
    return meta, per_core


def _build_program(cfg, meta):
    """Build the SPMD Bass program (same for all cores)."""
    from concourse import bacc

    NC, WIN, BINW = cfg.NC, cfg.WIN, cfg.BINW
    GSTEP = 8   # 1024 idxs per dma_gather: >1024 is over the Q7 scratch cap
    NQ = 4      # ucode MAX_SWDGE_QUEUES; rings process entries serially, so
                # spreading instructions over all 4 rings quadruples rate
    nc = bacc.Bacc("TRN2", target_bir_lowering=False, debug=False,
                   num_devices=NC, num_swdge_queues=NQ)

    # I/O
    xt = {}
    for sd, nm in ((0, "d"), (1, "s")):
        for h in (0, 1):
            xt[(sd, h)] = nc.dram_tensor(
                f"x_{nm}_h{h}", [cfg.HALF, 128], BF16, kind="ExternalInput")
    idx_d = nc.dram_tensor("idx", [128, meta["ICOLS"]], I16, kind="ExternalInput")
    msel_d = nc.dram_tensor("msel", [128, meta["MCOLS"]], BF16, kind="ExternalInput")
    wts_d = nc.dram_tensor("wts", [2, 2, 2, 128, 128], BF16, kind="ExternalInput")
    linwt_d = nc.dram_tensor("linwt", [128, cfg.OUT], BF16, kind="ExternalInput")
    bias1_d = nc.dram_tensor("bias1", [2, 128, 2 * 128], F32, kind="ExternalInput")
    bias2_d = nc.dram_tensor("bias2", [2, 128, 1], F32, kind="ExternalInput")
    linb_d = nc.dram_tensor("linb", [128, 2 * cfg.OUT], F32, kind="ExternalInput")
    out_d = nc.dram_tensor("out", [2 * cfg.S, cfg.OUT], F32, kind="ExternalOutput")

    z_loc = [nc.dram_tensor(f"z_loc{t}", [cfg.S, 128], BF16) for t in (0, 1)]
    z_full = [nc.dram_tensor(f"z_full{t}", [cfg.N, 128], BF16,
                             addr_space="Shared") for t in (0, 1)]
    z_hi = [nc.dram_tensor(f"z_hi{t}", [cfg.HALF, 128], BF16) for t in (0, 1)]

    calls, windows = meta["calls"], meta["windows"]
    call_lookup = meta["call_lookup"]

    gctr = [0]

    with tile.TileContext(nc) as tc:
        cpool = tc.alloc_tile_pool(name="const", bufs=1)
        gpool = tc.alloc_tile_pool(name="gather", bufs=4)
        ipool = tc.alloc_tile_pool(name="idx", bufs=8)
        mpool = tc.alloc_tile_pool(name="msel", bufs=2)
        apool = tc.alloc_tile_pool(name="aggs", bufs=2)
        zpool = tc.alloc_tile_pool(name="z", bufs=2)
        pagg = tc.alloc_tile_pool(name="pagg", bufs=2, space="PSUM")
        ptrf = tc.alloc_tile_pool(name="ptrf", bufs=2, space="PSUM")
        pfin = tc.alloc_tile_pool(name="pfin", bufs=2, space="PSUM")

        # constants to SBUF
        wsb = {}
        for l in (0, 1):
            for t in (0, 1):
                for ri in (0, 1):
                    w = cpool.tile([128, 128], BF16, tag=f"w{l}{t}{ri}", name=f"w_{l}{t}{ri}")
                    nc.sync.dma_start(w[:], wts_d[l, t, ri])
                    wsb[(l, t, ri)] = w
        linwt = cpool.tile([128, cfg.OUT], BF16, tag="linwt")
        nc.sync.dma_start(linwt[:], linwt_d[:])
        b1 = {}
        b2 = {}
        for t in (0, 1):
            b1[t] = cpool.tile([128, 256], F32, tag=f"b1{t}", name=f"b1_{t}")
            nc.sync.dma_start(b1[t][:], bias1_d[t])
            b2[t] = cpool.tile([128, 1], F32, tag=f"b2{t}", name=f"b2_{t}")
            nc.sync.dma_start(b2[t][:], bias2_d[t])
        linb = cpool.tile([128, 2 * cfg.OUT], F32, tag="linb")
        nc.sync.dma_start(linb[:], linb_d[:])

        def gather_tables(l, sd):
            if l == 0:
                return [xt[(sd, 0)].ap(), xt[(sd, 1)].ap()]
            return [z_full[sd][0:cfg.HALF, :], z_hi[sd].ap()]

        def do_layer(l, t):
            gtiles = {}
            if True:
                for w in range(cfg.NW):
                    ws = cfg.win_size(w)
                    g = w // cfg.GROUP
                    if w % cfg.GROUP == 0:
                        for ri in range(2):
                            for h in (0, 1):
                                call = calls[call_lookup[(t, g, ri, h)]]
                                C = call["nchunks"]
                                if C == 0:
                                    gtiles[(ri, h)] = None
                                    continue
                                it = ipool.tile([128, C * 8], I16, tag=f"i{ri}{h}", name=f"it{ri}{h}")
                                nc.sync.dma_start(
                                    it[:], idx_d[:, call["icol"]:call["icol"] + C * 8])
                                gt = gpool.tile([128, C, 128], BF16, tag=f"g{ri}{h}", name=f"gt{ri}{h}")
                                src = gather_tables(l, call["src_dis"])[h]
                                for j0 in range(0, C, GSTEP):
                                    cs = min(GSTEP, C - j0)
                                    nc.gpsimd.dma_gather(
                                        gt[:, j0:j0 + cs, :], src,
                                        it[:, j0 * 8:(j0 + cs) * 8],
                                        cs * 128, cs * 128, 128,
                                        queue_num=gctr[0] % NQ)
                                    gctr[0] += 1
                                gtiles[(ri, h)] = gt
                    wd = windows[(t, w)]
                    nch = len(wd["chunks"])
                    if nch:
                        mt = mpool.tile([128, nch * BINW], BF16, tag="m", name="mt")
                        for c0 in range(0, nch * BINW, 512):
                            c1 = min(c0 + 512, nch * BINW)
                            nc.sync.dma_start(
                                mt[:, c0:c1],
                                msel_d[:, wd["mcol"] + c0:wd["mcol"] + c1])
                    aggP = [pagg.tile([128, 256], F32, tag=f"agg{r}", name=f"aggP{r}") for r in (0, 1)]
                    nc.vector.memset(aggP[0][:], 0.0)
                    nc.vector.memset(aggP[1][:], 0.0)
                    last_of_r = {}
                    for ci, (ri, h, j, poff) in enumerate(wd["chunks"]):
                        last_of_r[ri] = ci
                    for ci, (ri, h, j, poff) in enumerate(wd["chunks"]):
                        gt = gtiles[(ri, h)]
                        nc.tensor.matmul(
                            aggP[ri][:, poff:poff + BINW],
                            gt[:, j, :],
                            mt[:, ci * BINW:(ci + 1) * BINW],
                            start=False, stop=(last_of_r[ri] == ci),
                            skip_group_check=True)
                    aggS = []
                    for r in (0, 1):
                        a = apool.tile([128, 256], BF16, tag=f"as{r}", name=f"aggS{r}")
                        nc.vector.tensor_copy(a[:, 0:ws], aggP[r][:, 0:ws])
                        aggS.append(a)
                    if l == 0:
                        outP = ptrf.tile([128, 256], F32, tag="tp")
                        nsub = (ws + 127) // 128
                        for j in range(nsub):
                            m = min(128, ws - j * 128)
                            nc.tensor.matmul(
                                outP[0:m, j * 128:j * 128 + 128],
                                aggS[0][:, j * 128:j * 128 + m],
                                wsb[(l, t, 0)][:], start=True, stop=False)
                            nc.tensor.matmul(
                                outP[0:m, j * 128:j * 128 + 128],
                                aggS[1][:, j * 128:j * 128 + m],
                                wsb[(l, t, 1)][:], start=False, stop=True)
                        tmp = zpool.tile([128, 256], F32, tag="tmp")
                        zsb = zpool.tile([128, 256], BF16, tag="zsb")
                        for j in range(nsub):
                            m = min(128, ws - j * 128)
                            sl = slice(j * 128, j * 128 + 128)
                            nc.vector.tensor_add(tmp[0:m, sl], outP[0:m, sl],
                                                 b1[t][0:m, sl])
                            nc.scalar.activation(
                                zsb[0:m, sl], tmp[0:m, sl],
                                mybir.ActivationFunctionType.Relu)
                            base = w * WIN + j * 128
                            nc.sync.dma_start(
                                z_loc[t][base:base + m, :], zsb[0:m, sl])
                    else:
                        z2P = ptrf.tile([128, 256], F32, tag="tp")
                        nc.tensor.matmul(z2P[:, 0:ws], wsb[(l, t, 0)][:],
                                         aggS[0][:, 0:ws], start=True, stop=False)
                        nc.tensor.matmul(z2P[:, 0:ws], wsb[(l, t, 1)][:],
                                         aggS[1][:, 0:ws], start=False, stop=True)
                        z2T = zpool.tile([128, 256], BF16, tag="z2t")
                        nc.scalar.activation(z2T[:, 0:ws], z2P[:, 0:ws],
                                             mybir.ActivationFunctionType.Relu,
                                             bias=b2[t][:])
                        fP = pfin.tile([128, 128], F32, tag="fp")
                        nsub = (ws + 127) // 128
                        for j in range(nsub):
                            m = min(128, ws - j * 128)
                            nc.tensor.matmul(
                                fP[0:m, j * 64:j * 64 + 64],
                                z2T[:, j * 128:j * 128 + m],
                                linwt[:], start=True, stop=True)
                        fo = zpool.tile([128, 128], F32, tag="fo")
                        for j in range(nsub):
                            m = min(128, ws - j * 128)
                            sl = slice(j * 64, j * 64 + 64)
                            nc.vector.tensor_add(fo[0:m, sl], fP[0:m, sl],
                                                 linb[0:m, sl])
                            base = t * cfg.S + w * WIN + j * 128
                            nc.sync.dma_start(out_d[base:base + m, :], fo[0:m, sl])

        # Interleave: each type's AllGather launches as soon as that type's
        # layer-1 windows are stored, overlapping the other type's compute.
        # Layer-2 gathers depend per-tensor on z_full/z_hi, so dd/ss chunks
        # start as soon as their own type's collective lands.
        for t in (0, 1):
            do_layer(0, t)
            nc.gpsimd.collective_compute(
                "AllGather", mybir.AluOpType.bypass,
                replica_groups=[list(range(NC))],
                ins=[z_loc[t].ap().opt()], outs=[z_full[t].ap().opt()])
            nc.sync.dma_start(z_hi[t].ap(), z_full[t][cfg.HALF:2 * cfg.HALF, :])
        for t in (0, 1):
            do_layer(1, t)

        for p in (pfin, ptrf, pagg, zpool, apool, mpool, ipool, gpool, cpool):
            p.release()

    nc.compile()
    return nc


def _make_inputs(cfg, per_core, x_drug, x_dis, Ws, bs, lin_w, lin_b):
    bf = ml_dtypes.bfloat16
    xb = {0: x_drug.astype(bf), 1: x_dis.astype(bf)}
    wts = np.zeros((2, 2, 2, 128, 128), np.float32)
    b1 = np.zeros((2, 128, 256), np.float32)
    b2 = np.zeros((2, 128, 1), np.float32)
    for l in (0, 1):
        for t in (0, 1):
            for ri in (0, 1):
                r = REL_OF_T[t][ri][0]
                wts[l, t, ri] = Ws[l, r]
            bsum = bs[l, REL_OF_T[t][0][0]] + bs[l, REL_OF_T[t][1][0]]
            if l == 0:
                b1[t] = np.tile(bsum[None, :], (128, 2))
            else:
                b2[t] = bsum[:, None].astype(np.float32)
    shared = {
        "wts": wts.astype(bf),
        "linwt": lin_w.T.astype(bf).copy(),
        "bias1": b1,
        "bias2": b2,
        "linb": np.tile(lin_b[None, :], (128, 2)).astype(np.float32),
    }
    for sd, nm in ((0, "d"), (1, "s")):
        for h in (0, 1):
            shared[f"x_{nm}_h{h}"] = np.ascontiguousarray(
                xb[sd][h * cfg.HALF:(h + 1) * cfg.HALF])
    in_maps = []
    for c in range(cfg.NC):
        m = dict(shared)
        m["idx"] = per_core[c]["idx"]
        m["msel"] = per_core[c]["msel"]
        in_maps.append(m)
    return in_maps


def run(cfg, x_drug, x_dis, eis, Ws, bs, lin_w, lin_b, trace=False):
    edge_arrays = {r: (eis[r][0].astype(np.int64), eis[r][1].astype(np.int64))
                   for r in range(4)}
    meta, per_core = _prep_graph(cfg, edge_arrays)
    nc = _build_program(cfg, meta)
    in_maps = _make_inputs(cfg, per_core, x_drug, x_dis, Ws, bs, lin_w, lin_b)
    res = run_bass_kernel_spmd(nc, in_maps, core_ids=list(range(cfg.NC)),
                               trace=trace)
    drug = np.zeros((cfg.N, cfg.OUT), np.float32)
    dis = np.zeros((cfg.N, cfg.OUT), np.float32)
    for c in range(cfg.NC):
        o = res.results[c]["out"]
        drug[c * cfg.S:(c + 1) * cfg.S] = o[:cfg.S]
        dis[c * cfg.S:(c + 1) * cfg.S] = o[cfg.S:]
    return (drug, dis), res


def kernel(x_drug, x_dis, ei_dd, ei_ss, ei_ds, ei_sd, Ws, bs, lin_w, lin_b):
    cfg = Cfg()
    eis = {0: np.asarray(ei_dd), 1: np.asarray(ei_ss),
           2: np.asarray(ei_ds), 3: np.asarray(ei_sd)}
    out, _ = run(cfg, np.asarray(x_drug), np.asarray(x_dis), eis,
                 np.asarray(Ws), np.asarray(bs),
                 np.asarray(lin_w), np.asarray(lin_b))
    return out



# revision 25
# speedup vs baseline: 1.3031x; 1.0137x over previous
"""HeteroGNN (2-layer hetero GCN) Trainium2 kernel, 8-core SPMD.

Strategy: destination-sharded. Each core owns 6250 drug + 6250 dis nodes.
Feature tables (bf16 rows) live in per-core HBM; edge gathers use
dma_gather (custom SWDGE row gather); scatter-add is done as one-hot
"Msel" matmuls accumulating in PSUM (edges chunked 128 at a time, each
chunk's destinations confined to a 32-wide bin so PSUM offsets are
program constants shared by all cores). Layer-1 output slices are
exchanged with two AllGather collectives, then layer 2 + final linear.
All graph preprocessing (degrees, norms, chunking, padding to the
max-over-cores schedule) happens on host in numpy.
"""

import numpy as np
import ml_dtypes

import sys

for _p in ("/opt/trn_rl_repo",):
    if _p not in sys.path:
        sys.path.insert(0, _p)

import concourse.bass as bass
import concourse.mybir as mybir
from concourse import tile
from concourse.bass_utils import run_bass_kernel_spmd

BF16 = mybir.dt.bfloat16
F32 = mybir.dt.float32
I16 = mybir.dt.int16


class Cfg:
    def __init__(self, n=50000, e=800000, ncores=8, win=256, binw=128, group=1):
        self.N = n              # nodes per type
        self.E = e              # edges per relation
        self.NC = ncores
        self.S = n // ncores    # dst nodes per core per type
        self.WIN = win          # dsts per PSUM window
        self.BINW = binw        # dsts per bin (fixed psum offset granularity)
        self.GROUP = group      # windows per gather call
        self.NW = (self.S + win - 1) // win   # windows per type
        self.NG = (self.NW + group - 1) // group
        self.HALF = n // 2      # rows per gather half-table (int16 idx limit)
        assert self.HALF <= 32768
        self.D = 128
        self.OUT = 64

    def win_size(self, w):
        return min(self.WIN, self.S - w * self.WIN)

    def nbins(self, w):
        ws = self.win_size(w)
        return (ws + self.BINW - 1) // self.BINW


# relations per dst type: (reference rel index, src_is_dis)
# drug dst: rel 0 (dd, src drug), rel 3 (sd, src dis)
# dis  dst: rel 1 (ss, src dis),  rel 2 (ds, src drug)
REL_OF_T = {0: [(0, 0), (3, 1)], 1: [(1, 1), (2, 0)]}
SELF_LOOP = {0: True, 1: True, 2: False, 3: False}


def _prep_graph(cfg, edge_arrays):
    """edge_arrays: dict rel_idx -> (row, col) int64 full edge lists.
    Returns (meta, per_core) where meta is the SPMD-uniform schedule and
    per_core[c] = dict(idx=int16 [128, ICOLS], msel=f32 [128, MCOLS])."""
    N, S, WIN, BINW, NC = cfg.N, cfg.S, cfg.WIN, cfg.BINW, cfg.NC

    # chunks[(t, w, r, h, b)][core] = list of (idx128 array, dloc array, norm array)
    group_chunks = {}
    for t in (0, 1):
        for ri, (r, src_dis) in enumerate(REL_OF_T[t]):
            row, col = edge_arrays[r]
            if SELF_LOOP[r]:
                sl = np.arange(N, dtype=np.int64)
                row = np.concatenate([row, sl])
                col = np.concatenate([col, sl])
            deg_s = np.bincount(row, minlength=N).astype(np.float64)
            deg_d = np.bincount(col, minlength=N).astype(np.float64)
            norm = (deg_s[row] ** -0.5 * deg_d[col] ** -0.5).astype(np.float32)
            core = col // S
            d_loc = col % S
            w = d_loc // WIN
            b = (d_loc % WIN) // BINW
            h = row // cfg.HALF
            idx16 = (row % cfg.HALF).astype(np.int16)
            # group key: (core, w, b, h)
            nb_max = (WIN + BINW - 1) // BINW
            key = ((core * cfg.NW + w) * nb_max + b) * 2 + h
            order = np.argsort(key, kind="stable")
            key_s = key[order]
            uk, starts = np.unique(key_s, return_index=True)
            starts = list(starts) + [len(key_s)]
            for gi, k in enumerate(uk):
                sl_ = order[starts[gi]:starts[gi + 1]]
                kk = int(k)
                hh = kk % 2
                kk //= 2
                bb = kk % nb_max
                kk //= nb_max
                ww = kk % cfg.NW
                cc = kk // cfg.NW
                gkey = (t, ww, ri, hh, bb)
                group_chunks.setdefault(gkey, {c: [] for c in range(NC)})
                lst = group_chunks[gkey][cc]
                for s0 in range(0, len(sl_), 128):
                    ee = sl_[s0:s0 + 128]
                    lst.append((idx16[ee], (d_loc[ee] % WIN) % BINW, norm[ee]))

    # C_max per slot key
    cmax = {}
    for gkey, bycore in group_chunks.items():
        cmax[gkey] = max(len(v) for v in bycore.values())

    # Build uniform schedule.
    # calls: per (t, g, ri, h): list of chunk slot keys in order (w asc, b asc, dup)
    calls = []           # (t, g, ri, h, src_dis, idx_col_off, nchunks)
    call_lookup = {}     # (t, g, ri, h) -> call index
    icol = 0
    for t in (0, 1):
        for g in range(cfg.NG):
            for ri in range(2):
                src_dis = REL_OF_T[t][ri][1]
                for h in (0, 1):
                    nch = 0
                    for w in range(g * cfg.GROUP, min((g + 1) * cfg.GROUP, cfg.NW)):
                        for b in range(cfg.nbins(w)):
                            nch += cmax.get((t, w, ri, h, b), 0)
                    call_lookup[(t, g, ri, h)] = len(calls)
                    calls.append(dict(t=t, g=g, ri=ri, h=h, src_dis=src_dis,
                                      icol=icol, nchunks=nch))
                    icol += nch * 8
    ICOLS = max(icol, 8)

    # windows: per (t, w): msel col offset + chunk list
    windows = {}
    mcol = 0
    for t in (0, 1):
        for w in range(cfg.NW):
            wch = []   # (ri, h, j_in_call, psum_off)
            # j_in_call accumulators per (ri, h) for this group
            for ri in range(2):
                for h in (0, 1):
                    j = 0
                    g = w // cfg.GROUP
                    for w2 in range(g * cfg.GROUP, w):
                        for b in range(cfg.nbins(w2)):
                            j += cmax.get((t, w2, ri, h, b), 0)
                    for b in range(cfg.nbins(w)):
                        for d in range(cmax.get((t, w, ri, h, b), 0)):
                            wch.append((ri, h, j, b * BINW))
                            j += 1
            windows[(t, w)] = dict(mcol=mcol, chunks=wch)
            mcol += len(wch) * BINW
    MCOLS = max(mcol, BINW)

    meta = dict(calls=calls, call_lookup=call_lookup, windows=windows,
                ICOLS=ICOLS, MCOLS=MCOLS)

    # Per-core data arrays
    per_core = []
    for c in range(NC):
        idxa = np.zeros((16, ICOLS), np.int16)
        msel = np.zeros((128, MCOLS), np.float32)
        for call in calls:
            t, g, ri, h = call["t"], call["g"], call["ri"], call["h"]
            j = 0
            for w in range(g * cfg.GROUP, min((g + 1) * cfg.GROUP, cfg.NW)):
                for b in range(cfg.nbins(w)):
                    ck = group_chunks.get((t, w, ri, h, b))
                    lst = ck[c] if ck else []
                    for d in range(cmax.get((t, w, ri, h, b), 0)):
                        if d < len(lst):
                            ii, dd, nn = lst[d]
                            s0 = j * 128
                            sl = np.arange(s0, s0 + len(ii))
                            idxa[sl % 16, call["icol"] + sl // 16] = ii
                        j += 1
        # msel fill: walk windows
        for (t, w), wd in windows.items():
            dupc = {}
            for ci, (ri, h, jc, poff) in enumerate(wd["chunks"]):
                b = poff // BINW
                ck = group_chunks.get((t, w, ri, h, b))
                lst = ck[c] if ck else []
                d = dupc.get((ri, h, poff), 0)
                dupc[(ri, h, poff)] = d + 1
                if d < len(lst):
                    ii, dd, nn = lst[d]
                    m0 = wd["mcol"] + ci * BINW
                    msel[np.arange(len(ii)), m0 + dd] = nn
        idx_full = np.tile(idxa, (8, 1))
        per_core.append(dict(idx=idx_full,
                             msel=msel.astype(ml_dtypes.bfloat16)))
    return meta, per_core


def _build_program(cfg, meta):
    """Build the SPMD Bass program (same for all cores)."""
    from concourse import bacc

    NC, WIN, BINW = cfg.NC, cfg.WIN, cfg.BINW
    GSTEP = 8   # 1024 idxs per dma_gather: >1024 is over the Q7 scratch cap
    NQ = 4      # ucode MAX_SWDGE_QUEUES; rings process entries serially, so
                # spreading instructions over all 4 rings quadruples rate
    nc = bacc.Bacc("TRN2", target_bir_lowering=False, debug=False,
                   num_devices=NC, num_swdge_queues=NQ)

    # I/O
    xt = {}
    for sd, nm in ((0, "d"), (1, "s")):
        for h in (0, 1):
            xt[(sd, h)] = nc.dram_tensor(
                f"x_{nm}_h{h}", [cfg.HALF, 128], BF16, kind="ExternalInput")
    idx_d = nc.dram_tensor("idx", [128, meta["ICOLS"]], I16, kind="ExternalInput")
    msel_d = nc.dram_tensor("msel", [128, meta["MCOLS"]], BF16, kind="ExternalInput")
    wts_d = nc.dram_tensor("wts", [2, 2, 2, 128, 128], BF16, kind="ExternalInput")
    linwt_d = nc.dram_tensor("linwt", [128, cfg.OUT], BF16, kind="ExternalInput")
    bias1_d = nc.dram_tensor("bias1", [2, 128, 2 * 128], F32, kind="ExternalInput")
    bias2_d = nc.dram_tensor("bias2", [2, 128, 1], F32, kind="ExternalInput")
    linb_d = nc.dram_tensor("linb", [128, 2 * cfg.OUT], F32, kind="ExternalInput")
    out_d = nc.dram_tensor("out", [2 * cfg.S, cfg.OUT], F32, kind="ExternalOutput")

    z_loc = [nc.dram_tensor(f"z_loc{t}", [cfg.S, 128], BF16) for t in (0, 1)]
    z_full = [nc.dram_tensor(f"z_full{t}", [cfg.N, 128], BF16,
                             addr_space="Shared") for t in (0, 1)]
    z_hi = [nc.dram_tensor(f"z_hi{t}", [cfg.HALF, 128], BF16) for t in (0, 1)]

    calls, windows = meta["calls"], meta["windows"]
    call_lookup = meta["call_lookup"]

    gctr = [0]

    with tile.TileContext(nc) as tc:
        cpool = tc.alloc_tile_pool(name="const", bufs=1)
        gpool = tc.alloc_tile_pool(name="gather", bufs=4)
        ipool = tc.alloc_tile_pool(name="idx", bufs=8)
        mpool = tc.alloc_tile_pool(name="msel", bufs=2)
        apool = tc.alloc_tile_pool(name="aggs", bufs=2)
        zpool = tc.alloc_tile_pool(name="z", bufs=2)
        pagg = tc.alloc_tile_pool(name="pagg", bufs=2, space="PSUM")
        ptrf = tc.alloc_tile_pool(name="ptrf", bufs=2, space="PSUM")
        pfin = tc.alloc_tile_pool(name="pfin", bufs=2, space="PSUM")

        # constants to SBUF
        wsb = {}
        for l in (0, 1):
            for t in (0, 1):
                for ri in (0, 1):
                    w = cpool.tile([128, 128], BF16, tag=f"w{l}{t}{ri}", name=f"w_{l}{t}{ri}")
                    nc.sync.dma_start(w[:], wts_d[l, t, ri])
                    wsb[(l, t, ri)] = w
        linwt = cpool.tile([128, cfg.OUT], BF16, tag="linwt")
        nc.sync.dma_start(linwt[:], linwt_d[:])
        b1 = {}
        b2 = {}
        for t in (0, 1):
            b1[t] = cpool.tile([128, 256], F32, tag=f"b1{t}", name=f"b1_{t}")
            nc.sync.dma_start(b1[t][:], bias1_d[t])
            b2[t] = cpool.tile([128, 1], F32, tag=f"b2{t}", name=f"b2_{t}")
            nc.sync.dma_start(b2[t][:], bias2_d[t])
        linb = cpool.tile([128, 2 * cfg.OUT], F32, tag="linb")
        nc.sync.dma_start(linb[:], linb_d[:])

        def gather_tables(l, sd):
            if l == 0:
                return [xt[(sd, 0)].ap(), xt[(sd, 1)].ap()]
            return [z_full[sd][0:cfg.HALF, :], z_hi[sd].ap()]

        def do_layer(l, t):
            gtiles = {}
            if True:
                for w in range(cfg.NW):
                    ws = cfg.win_size(w)
                    g = w // cfg.GROUP
                    if w % cfg.GROUP == 0:
                        for ri in range(2):
                            for h in (0, 1):
                                call = calls[call_lookup[(t, g, ri, h)]]
                                C = call["nchunks"]
                                if C == 0:
                                    gtiles[(ri, h)] = None
                                    continue
                                it = ipool.tile([128, C * 8], I16, tag=f"i{ri}{h}", name=f"it{ri}{h}")
                                nc.sync.dma_start(
                                    it[:], idx_d[:, call["icol"]:call["icol"] + C * 8])
                                gt = gpool.tile([128, C, 128], BF16, tag=f"g{ri}{h}", name=f"gt{ri}{h}")
                                src = gather_tables(l, call["src_dis"])[h]
                                for j0 in range(0, C, GSTEP):
                                    cs = min(GSTEP, C - j0)
                                    nc.gpsimd.dma_gather(
                                        gt[:, j0:j0 + cs, :], src,
                                        it[:, j0 * 8:(j0 + cs) * 8],
                                        cs * 128, cs * 128, 128,
                                        queue_num=gctr[0] % NQ)
                                    gctr[0] += 1
                                gtiles[(ri, h)] = gt
                    wd = windows[(t, w)]
                    nch = len(wd["chunks"])
                    if nch:
                        mt = mpool.tile([128, nch * BINW], BF16, tag="m", name="mt")
                        for c0 in range(0, nch * BINW, 512):
                            c1 = min(c0 + 512, nch * BINW)
                            nc.sync.dma_start(
                                mt[:, c0:c1],
                                msel_d[:, wd["mcol"] + c0:wd["mcol"] + c1])
                    aggP = [pagg.tile([128, 256], F32, tag=f"agg{r}", name=f"aggP{r}") for r in (0, 1)]
                    nc.vector.memset(aggP[0][:], 0.0)
                    nc.vector.memset(aggP[1][:], 0.0)
                    last_of_r = {}
                    for ci, (ri, h, j, poff) in enumerate(wd["chunks"]):
                        last_of_r[ri] = ci
                    for ci, (ri, h, j, poff) in enumerate(wd["chunks"]):
                        gt = gtiles[(ri, h)]
                        nc.tensor.matmul(
                            aggP[ri][:, poff:poff + BINW],
                            gt[:, j, :],
                            mt[:, ci * BINW:(ci + 1) * BINW],
                            start=False, stop=(last_of_r[ri] == ci),
                            skip_group_check=True)
                    aggS = []
                    for r in (0, 1):
                        a = apool.tile([128, 256], BF16, tag=f"as{r}", name=f"aggS{r}")
                        nc.vector.tensor_copy(a[:, 0:ws], aggP[r][:, 0:ws])
                        aggS.append(a)
                    if l == 0:
                        outP = ptrf.tile([128, 256], F32, tag="tp")
                        nsub = (ws + 127) // 128
                        for j in range(nsub):
                            m = min(128, ws - j * 128)
                            nc.tensor.matmul(
                                outP[0:m, j * 128:j * 128 + 128],
                                aggS[0][:, j * 128:j * 128 + m],
                                wsb[(l, t, 0)][:], start=True, stop=False)
                            nc.tensor.matmul(
                                outP[0:m, j * 128:j * 128 + 128],
                                aggS[1][:, j * 128:j * 128 + m],
                                wsb[(l, t, 1)][:], start=False, stop=True)
                        tmp = zpool.tile([128, 256], F32, tag="tmp")
                        zsb = zpool.tile([128, 256], BF16, tag="zsb")
                        for j in range(nsub):
                            m = min(128, ws - j * 128)
                            sl = slice(j * 128, j * 128 + 128)
                            nc.vector.tensor_add(tmp[0:m, sl], outP[0:m, sl],
                                                 b1[t][0:m, sl])
                            nc.scalar.activation(
                                zsb[0:m, sl], tmp[0:m, sl],
                                mybir.ActivationFunctionType.Relu)
                            base = w * WIN + j * 128
                            nc.sync.dma_start(
                                z_loc[t][base:base + m, :], zsb[0:m, sl])
                    else:
                        z2P = ptrf.tile([128, 256], F32, tag="tp")
                        nc.tensor.matmul(z2P[:, 0:ws], wsb[(l, t, 0)][:],
                                         aggS[0][:, 0:ws], start=True, stop=False)
                        nc.tensor.matmul(z2P[:, 0:ws], wsb[(l, t, 1)][:],
                                         aggS[1][:, 0:ws], start=False, stop=True)
                        z2T = zpool.tile([128, 256], BF16, tag="z2t")
                        nc.scalar.activation(z2T[:, 0:ws], z2P[:, 0:ws],
                                             mybir.ActivationFunctionType.Relu,
                                             bias=b2[t][:])
                        fP = pfin.tile([128, 128], F32, tag="fp")
                        nsub = (ws + 127) // 128
                        for j in range(nsub):
                            m = min(128, ws - j * 128)
                            nc.tensor.matmul(
                                fP[0:m, j * 64:j * 64 + 64],
                                z2T[:, j * 128:j * 128 + m],
                                linwt[:], start=True, stop=True)
                        fo = zpool.tile([128, 128], F32, tag="fo")
                        for j in range(nsub):
                            m = min(128, ws - j * 128)
                            sl = slice(j * 64, j * 64 + 64)
                            nc.vector.tensor_add(fo[0:m, sl], fP[0:m, sl],
                                                 linb[0:m, sl])
                            base = t * cfg.S + w * WIN + j * 128
                            nc.sync.dma_start(out_d[base:base + m, :], fo[0:m, sl])

        # Interleave: each type's AllGather launches as soon as that type's
        # layer-1 windows are stored, overlapping the other type's compute.
        # Layer-2 gathers depend per-tensor on z_full/z_hi, so dd/ss chunks
        # start as soon as their own type's collective lands.
        do_layer(0, 0)
        do_layer(0, 1)
        for t in (0, 1):
            nc.gpsimd.collective_compute(
                "AllGather", mybir.AluOpType.bypass,
                replica_groups=[list(range(NC))],
                ins=[z_loc[t].ap().opt()], outs=[z_full[t].ap().opt()])
            nc.sync.dma_start(z_hi[t].ap(), z_full[t][cfg.HALF:2 * cfg.HALF, :])
        for t in (0, 1):
            do_layer(1, t)

        for p in (pfin, ptrf, pagg, zpool, apool, mpool, ipool, gpool, cpool):
            p.release()

    nc.compile()
    return nc


def _make_inputs(cfg, per_core, x_drug, x_dis, Ws, bs, lin_w, lin_b):
    bf = ml_dtypes.bfloat16
    xb = {0: x_drug.astype(bf), 1: x_dis.astype(bf)}
    wts = np.zeros((2, 2, 2, 128, 128), np.float32)
    b1 = np.zeros((2, 128, 256), np.float32)
    b2 = np.zeros((2, 128, 1), np.float32)
    for l in (0, 1):
        for t in (0, 1):
            for ri in (0, 1):
                r = REL_OF_T[t][ri][0]
                wts[l, t, ri] = Ws[l, r]
            bsum = bs[l, REL_OF_T[t][0][0]] + bs[l, REL_OF_T[t][1][0]]
            if l == 0:
                b1[t] = np.tile(bsum[None, :], (128, 2))
            else:
                b2[t] = bsum[:, None].astype(np.float32)
    shared = {
        "wts": wts.astype(bf),
        "linwt": lin_w.T.astype(bf).copy(),
        "bias1": b1,
        "bias2": b2,
        "linb": np.tile(lin_b[None, :], (128, 2)).astype(np.float32),
    }
    for sd, nm in ((0, "d"), (1, "s")):
        for h in (0, 1):
            shared[f"x_{nm}_h{h}"] = np.ascontiguousarray(
                xb[sd][h * cfg.HALF:(h + 1) * cfg.HALF])
    in_maps = []
    for c in range(cfg.NC):
        m = dict(shared)
        m["idx"] = per_core[c]["idx"]
        m["msel"] = per_core[c]["msel"]
        in_maps.append(m)
    return in_maps


def run(cfg, x_drug, x_dis, eis, Ws, bs, lin_w, lin_b, trace=False):
    edge_arrays = {r: (eis[r][0].astype(np.int64), eis[r][1].astype(np.int64))
                   for r in range(4)}
    meta, per_core = _prep_graph(cfg, edge_arrays)
    nc = _build_program(cfg, meta)
    in_maps = _make_inputs(cfg, per_core, x_drug, x_dis, Ws, bs, lin_w, lin_b)
    res = run_bass_kernel_spmd(nc, in_maps, core_ids=list(range(cfg.NC)),
                               trace=trace)
    drug = np.zeros((cfg.N, cfg.OUT), np.float32)
    dis = np.zeros((cfg.N, cfg.OUT), np.float32)
    for c in range(cfg.NC):
        o = res.results[c]["out"]
        drug[c * cfg.S:(c + 1) * cfg.S] = o[:cfg.S]
        dis[c * cfg.S:(c + 1) * cfg.S] = o[cfg.S:]
    return (drug, dis), res


def kernel(x_drug, x_dis, ei_dd, ei_ss, ei_ds, ei_sd, Ws, bs, lin_w, lin_b):
    cfg = Cfg()
    eis = {0: np.asarray(ei_dd), 1: np.asarray(ei_ss),
           2: np.asarray(ei_ds), 3: np.asarray(ei_sd)}
    out, _ = run(cfg, np.asarray(x_drug), np.asarray(x_dis), eis,
                 np.asarray(Ws), np.asarray(bs),
                 np.asarray(lin_w), np.asarray(lin_b))
    return out

